# revision 1
# baseline (speedup 1.0000x reference)
"""Bass/TRN2 kernel for nn_EnvCollLoss (oriented-footprint raster collision loss).

Strategy: agents are sharded by map index across 8 cores (2 cores per map).
Each core holds its map as a Y8-bitpacked fp16 ap_gather table in SBUF
(partition j holds raster columns x===j mod 16; index e=(ix//16)*128+(iy//16)
returns the two 8-row packed words covering rows [16*(iy//16), +16)).
Per point: gather word-pair + a one-hot pair (by ix%16), mask-multiply,
block-diagonal matmul reduces the 16 candidate partitions, transpose-DMA back
to state-major, integer bit-extract, masked argmin via Max/MaxIndex, penalty.
"""
import sys
import types
import numpy as np
from contextlib import ExitStack

NA, T = 256, 100
N_MAPS, MAP_H, MAP_W = 4, 2048, 2048
PU, PV = 10, 20
P = PU * PV  # 200
N_CORES = 8

# jnp.linspace(-0.5, 0.5, 10/20, dtype=float32) exact values (validated vs jax)
_UU10 = np.array([-0.5, -0.3888889, -0.2777778, -0.16666667, -0.05555556,
                  0.05555556, 0.16666667, 0.2777778, 0.3888889, 0.5], dtype=np.float32)
_VV20 = np.linspace(-0.5, 0.5, 20, dtype=np.float32)


def _install_ntff_hook():
    import antenv
    if "antenv.axon_hooks" in sys.modules:
        return
    try:
        from trn_agent_boot.trn_boot import _ntff_profile_via_ctypes
        hook = _ntff_profile_via_ctypes("/opt/axon/libaxon_pjrt.so")
    except Exception:
        hook = None
    mod = types.ModuleType("antenv.axon_hooks")
    mod._hook = hook
    mod.get_axon_ntff_profile_hook = lambda: mod._hook
    mod.set_axon_ntff_profile_hook = lambda h: setattr(mod, "_hook", h)
    sys.modules["antenv.axon_hooks"] = mod
    antenv.axon_hooks = mod


_PROGRAM_CACHE = {}


def _build_program(n_tiles):
    import concourse.tile as tile
    from concourse import bacc, mybir

    dt = mybir.dt
    A = mybir.AluOpType

    nc = bacc.Bacc("TRN2", target_bir_lowering=False, debug=False,
                   enable_asserts=False, num_devices=N_CORES)
    S = n_tiles * 128

    tab_in = nc.dram_tensor("tab", [128, 16384 * 2], dt.float16, kind="ExternalInput").ap()
    mt_in = nc.dram_tensor("mt", [128, 32 * 2], dt.float16, kind="ExternalInput").ap()
    ones_in = nc.dram_tensor("ones8", [128, 8], dt.float16, kind="ExternalInput").ap()
    uu_in = nc.dram_tensor("uu", [128, P], dt.float32, kind="ExternalInput").ap()
    vv_in = nc.dram_tensor("vv", [128, P], dt.float32, kind="ExternalInput").ap()
    io_in = nc.dram_tensor("iotaf", [128, P], dt.float32, kind="ExternalInput").ap()
    dx_in = nc.dram_tensor("dxrep", [128, 1], dt.float32, kind="ExternalInput").ap()
    traj_in = nc.dram_tensor("trajsh", [S, 4], dt.float32, kind="ExternalInput").ap()
    att_in = nc.dram_tensor("attsh", [S, 2], dt.float32, kind="ExternalInput").ap()
    out_dram = nc.dram_tensor("outsh", [n_tiles, 128], dt.float32, kind="ExternalOutput").ap()

    with tile.TileContext(nc) as tc, ExitStack() as ctx:
        cpool = ctx.enter_context(tc.tile_pool(name="const", bufs=1))
        wpool = ctx.enter_context(tc.tile_pool(name="work", bufs=2))
        gpool = ctx.enter_context(tc.tile_pool(name="gath", bufs=2))
        spool = ctx.enter_context(tc.tile_pool(name="stgp", bufs=1))
        ppool = ctx.enter_context(tc.tile_pool(name="ps", bufs=8, space="PSUM"))

        tab = cpool.tile([128, 16384 * 2], dt.float16)
        nc.sync.dma_start(tab[:], tab_in)
        mt = cpool.tile([128, 64], dt.float16)
        nc.sync.dma_start(mt[:], mt_in)
        ones8 = cpool.tile([128, 8], dt.float16)
        nc.sync.dma_start(ones8[:], ones_in)
        uu = cpool.tile([128, P], dt.float32)
        nc.sync.dma_start(uu[:], uu_in)
        vv = cpool.tile([128, P], dt.float32)
        nc.sync.dma_start(vv[:], vv_in)
        iotaf = cpool.tile([128, P], dt.float32)
        nc.sync.dma_start(iotaf[:], io_in)
        dxrep = cpool.tile([128, 1], dt.float32)
        nc.sync.dma_start(dxrep[:], dx_in)
        invdx = cpool.tile([128, 1], dt.float32)
        nc.vector.reciprocal(invdx[:], dxrep[:])

        for it in range(n_tiles):
            tr = wpool.tile([128, 4], dt.float32, tag="tr")
            nc.sync.dma_start(tr[:], traj_in[it * 128:(it + 1) * 128, :])
            at = wpool.tile([128, 2], dt.float32, tag="at")
            nc.sync.dma_start(at[:], att_in[it * 128:(it + 1) * 128, :])
            cx, cy = tr[:, 0:1], tr[:, 1:2]
            hx0, hy0 = tr[:, 2:3], tr[:, 3:4]
            Lat, Wat = at[:, 0:1], at[:, 1:2]

            # den = sqrt(hx0^2+hy0^2) via near-1 closed form; hn = h * recip(den)
            ps = wpool.tile([128, 8], dt.float32, tag="ps")
            nc.vector.tensor_tensor(ps[:, 0:1], hx0, hx0, A.mult)
            nc.vector.tensor_tensor(ps[:, 1:2], hy0, hy0, A.mult)
            nc.vector.tensor_tensor(ps[:, 2:3], ps[:, 0:1], ps[:, 1:2], A.add)  # x
            nc.vector.tensor_scalar(ps[:, 3:4], ps[:, 2:3], -1.0, None, A.add)  # delta
            nc.vector.tensor_scalar(ps[:, 4:5], ps[:, 3:4], 0.5, None, A.mult)
            nc.vector.tensor_tensor(ps[:, 5:6], ps[:, 3:4], ps[:, 3:4], A.mult)
            nc.vector.tensor_scalar(ps[:, 5:6], ps[:, 5:6], -0.125, None, A.mult)
            nc.vector.tensor_tensor(ps[:, 4:5], ps[:, 4:5], ps[:, 5:6], A.add)
            nc.vector.tensor_scalar(ps[:, 4:5], ps[:, 4:5], 1.0, None, A.add)   # den
            inv = wpool.tile([128, 1], dt.float32, tag="inv")
            nc.vector.reciprocal(inv[:], ps[:, 4:5])
            hh = wpool.tile([128, 4], dt.float32, tag="hh")
            nc.vector.tensor_scalar(hh[:, 0:1], hx0, inv[:], None, A.mult)   # hx
            nc.vector.tensor_scalar(hh[:, 1:2], hy0, inv[:], None, A.mult)   # hy
            nc.vector.tensor_scalar(hh[:, 2:3], hh[:, 1:2], -1.0, None, A.mult)  # -hy

            bu = wpool.tile([128, P], dt.float32, tag="bu")
            nc.vector.tensor_scalar(bu[:], uu[:], Lat, None, A.mult)
            bv = wpool.tile([128, P], dt.float32, tag="bv")
            nc.vector.tensor_scalar(bv[:], vv[:], Wat, None, A.mult)
            t1 = wpool.tile([128, P], dt.float32, tag="t1")
            nc.vector.tensor_scalar(t1[:], bu[:], hh[:, 0:1], None, A.mult)
            ox = wpool.tile([128, P], dt.float32, tag="ox")
            nc.vector.scalar_tensor_tensor(ox[:], bv[:], hh[:, 2:3], t1[:], A.mult, A.add)
            nc.vector.tensor_scalar(t1[:], bu[:], hh[:, 1:2], None, A.mult)
            oy = wpool.tile([128, P], dt.float32, tag="oy")
            nc.vector.scalar_tensor_tensor(oy[:], bv[:], hh[:, 0:1], t1[:], A.mult, A.add)

            d2 = wpool.tile([128, P], dt.float32, tag="d2")
            nc.vector.tensor_tensor(d2[:], ox[:], ox[:], A.mult)
            nc.vector.tensor_tensor(t1[:], oy[:], oy[:], A.mult)
            nc.vector.tensor_tensor(d2[:], d2[:], t1[:], A.add)

            ixi = wpool.tile([128, P], dt.int32, tag="ixi")
            iyi = wpool.tile([128, P], dt.int32, tag="iyi")
            for (ov, ctr, res) in ((ox, cx, ixi), (oy, cy, iyi)):
                pw = wpool.tile([128, P], dt.float32, tag="pw")
                nc.vector.tensor_scalar(pw[:], ov[:], ctr, None, A.add)      # px
                nc.vector.tensor_scalar(pw[:], pw[:], invdx[:], None, A.mult)
                nc.vector.tensor_scalar(pw[:], pw[:], 0.0, 2047.0, A.max, A.min)
                ci = wpool.tile([128, P], dt.int32, tag="ci")
                nc.vector.tensor_copy(ci[:], pw[:])                          # RNE
                cf = wpool.tile([128, P], dt.float32, tag="cf")
                nc.vector.tensor_copy(cf[:], ci[:])
                ad = wpool.tile([128, P], dt.float32, tag="ad")
                nc.vector.tensor_tensor(ad[:], cf[:], pw[:], A.is_gt)
                adi = wpool.tile([128, P], dt.int32, tag="adi")
                nc.vector.tensor_copy(adi[:], ad[:])
                nc.vector.tensor_tensor(res[:], ci[:], adi[:], A.subtract)

            x16 = wpool.tile([128, P], dt.int32, tag="x16")
            nc.vector.tensor_scalar(x16[:], ixi[:], 4, None, A.logical_shift_right)
            jst = wpool.tile([128, P], dt.int32, tag="jst")
            nc.vector.tensor_scalar(jst[:], ixi[:], 1, 30, A.logical_shift_left, A.bitwise_and)
            jst16 = wpool.tile([128, P], dt.int16, tag="jst16")
            nc.vector.tensor_copy(jst16[:], jst[:])
            y8p = wpool.tile([128, P], dt.int32, tag="y8p")
            nc.vector.tensor_scalar(y8p[:], iyi[:], 4, None, A.logical_shift_right)
            e32 = wpool.tile([128, P], dt.int32, tag="e32")
            nc.vector.scalar_tensor_tensor(e32[:], x16[:], 128, y8p[:], A.mult, A.add)
            e16 = wpool.tile([128, P], dt.int16, tag="e16")
            nc.vector.tensor_copy(e16[:], e32[:])
            ssel = wpool.tile([128, P], dt.int32, tag="ssel")
            nc.vector.tensor_scalar(ssel[:], iyi[:], 3, 1, A.logical_shift_right, A.bitwise_and)
            sself = wpool.tile([128, P], dt.float32, tag="sself")
            nc.vector.tensor_copy(sself[:], ssel[:])
            rbit = wpool.tile([128, P], dt.int32, tag="rbit")
            nc.vector.tensor_scalar(rbit[:], iyi[:], 7, None, A.bitwise_and)

            g1 = gpool.tile([128, 16 * P * 2], dt.float16, tag="g1")
            nc.gpsimd.ap_gather(g1[:], tab[:], e16[:], channels=128,
                                num_elems=16384, d=2, num_idxs=16 * P)
            g2 = gpool.tile([128, 16 * P * 2], dt.float16, tag="g2")
            nc.gpsimd.ap_gather(g2[:], mt[:], jst16[:], channels=128,
                                num_elems=32, d=2, num_idxs=16 * P)
            nc.vector.tensor_tensor(g1[:], g1[:], g2[:], A.mult)

            # block-diagonal reduce: psum[g, i] = sum_{k in group g} g1[k, i]
            wsel = wpool.tile([128, P * 2], dt.float32, tag="wsel")
            stg = spool.tile([8, 16 * P * 2], dt.float32, tag="stg")
            CH = 400
            for c in range(0, 16 * P * 2, CH):
                pt = ppool.tile([8, CH], dt.float32, tag="pt")
                nc.tensor.matmul(pt[:], ones8[:], g1[:, c:c + CH], start=True, stop=True)
                nc.scalar.copy(stg[:, c:c + CH], pt[:])
            # repartition: stg[g, (s*16+j)*2+b] -> wsel[16g+j, s*2+b]
            src4 = stg[:].rearrange("g (s j b) -> g j s b", s=P, j=16, b=2)
            dst4 = wsel[:].rearrange("(gg j) f -> gg j f", gg=8, j=16)
            for j in range(16):
                nc.sync.dma_start(dst4[:, j, :], src4[:, j, :, :])

            # s-select word pair -> w; integer bit extract
            w0 = wsel[:].rearrange("p (s b) -> p s b", b=2)[:, :, 0:1].rearrange("p s b -> p (s b)")
            w1 = wsel[:].rearrange("p (s b) -> p s b", b=2)[:, :, 1:2].rearrange("p s b -> p (s b)")
            wd = wpool.tile([128, P], dt.float32, tag="wd")
            nc.vector.tensor_tensor(wd[:], w1, w0, A.subtract)
            nc.vector.tensor_tensor(wd[:], wd[:], sself[:], A.mult)
            nc.vector.tensor_tensor(wd[:], wd[:], w0, A.add)
            wi = wpool.tile([128, P], dt.int32, tag="wi")
            nc.vector.tensor_copy(wi[:], wd[:])
            nc.vector.tensor_tensor(wi[:], wi[:], rbit[:], A.logical_shift_right)
            nc.vector.tensor_scalar(wi[:], wi[:], 1, None, A.bitwise_and)

            key = wpool.tile([128, P], dt.float32, tag="key")
            nc.vector.scalar_tensor_tensor(key[:], wi[:], 1e30, d2[:], A.mult, A.add)
            nc.vector.tensor_scalar(key[:], key[:], -1.0, None, A.mult)
            mx8 = wpool.tile([128, 8], dt.float32, tag="mx8")
            nc.vector.max(mx8[:], key[:])
            mi8 = wpool.tile([128, 8], dt.uint32, tag="mi8")
            nc.vector.max_index(mi8[:], mx8[:], key[:])

            mk = wpool.tile([128, 1], dt.float32, tag="mk")
            nc.vector.tensor_scalar(mk[:], mx8[:, 0:1], -1.0, None, A.mult)
            idxf = wpool.tile([128, 1], dt.float32, tag="idxf")
            nc.vector.tensor_copy(idxf[:], mi8[:, 0:1])
            sel = wpool.tile([128, P], dt.float32, tag="sel")
            nc.vector.tensor_scalar(sel[:], iotaf[:], idxf[:], None, A.is_equal)
            oxs = wpool.tile([128, 1], dt.float32, tag="oxs")
            nc.vector.scalar_tensor_tensor(t1[:], ox[:], 1.0, sel[:], A.mult, A.mult,
                                           accum_out=oxs[:])
            oys = wpool.tile([128, 1], dt.float32, tag="oys")
            nc.vector.scalar_tensor_tensor(t1[:], oy[:], 1.0, sel[:], A.mult, A.mult,
                                           accum_out=oys[:])

            fin = wpool.tile([128, 12], dt.float32, tag="fin")
            nc.vector.tensor_tensor(fin[:, 0:1], cx, oxs[:], A.add)       # cxs
            nc.vector.tensor_tensor(fin[:, 1:2], cy, oys[:], A.add)
            nc.vector.tensor_tensor(fin[:, 0:1], cx, fin[:, 0:1], A.subtract)  # dxx
            nc.vector.tensor_tensor(fin[:, 1:2], cy, fin[:, 1:2], A.subtract)
            nc.vector.tensor_tensor(fin[:, 0:1], fin[:, 0:1], fin[:, 0:1], A.mult)
            nc.vector.tensor_tensor(fin[:, 1:2], fin[:, 1:2], fin[:, 1:2], A.mult)
            nc.vector.tensor_tensor(fin[:, 2:3], fin[:, 0:1], fin[:, 1:2], A.add)  # w2
            # dist = sqrt(w2): act-sqrt + one Newton step with exact recip
            nc.scalar.activation(fin[:, 3:4], fin[:, 2:3], mybir.ActivationFunctionType.Sqrt)
            nc.vector.reciprocal(fin[:, 4:5], fin[:, 3:4])
            nc.vector.tensor_tensor(fin[:, 4:5], fin[:, 2:3], fin[:, 4:5], A.mult)
            nc.vector.tensor_tensor(fin[:, 4:5], fin[:, 4:5], fin[:, 3:4], A.add)
            nc.vector.tensor_scalar(fin[:, 3:4], fin[:, 4:5], 0.5, None, A.mult)   # dist
            # pen = sqrt(L^2/4 + W^2/4), same refinement
            nc.vector.tensor_tensor(fin[:, 5:6], Lat, Lat, A.mult)
            nc.vector.tensor_scalar(fin[:, 5:6], fin[:, 5:6], 0.25, None, A.mult)
            nc.vector.tensor_tensor(fin[:, 6:7], Wat, Wat, A.mult)
            nc.vector.tensor_scalar(fin[:, 6:7], fin[:, 6:7], 0.25, None, A.mult)
            nc.vector.tensor_tensor(fin[:, 5:6], fin[:, 5:6], fin[:, 6:7], A.add)
            nc.scalar.activation(fin[:, 6:7], fin[:, 5:6], mybir.ActivationFunctionType.Sqrt)
            nc.vector.reciprocal(fin[:, 7:8], fin[:, 6:7])
            nc.vector.tensor_tensor(fin[:, 7:8], fin[:, 5:6], fin[:, 7:8], A.mult)
            nc.vector.tensor_tensor(fin[:, 7:8], fin[:, 7:8], fin[:, 6:7], A.add)
            nc.vector.tensor_scalar(fin[:, 6:7], fin[:, 7:8], 0.5, None, A.mult)   # pen
            nc.vector.reciprocal(fin[:, 7:8], fin[:, 6:7])
            nc.vector.tensor_tensor(fin[:, 8:9], fin[:, 3:4], fin[:, 7:8], A.mult)
            nc.vector.tensor_scalar(fin[:, 8:9], fin[:, 8:9], -1.0, 1.0, A.mult, A.add)
            nc.vector.tensor_scalar(fin[:, 9:10], mk[:], 1e29, None, A.is_lt)
            nc.vector.tensor_tensor(fin[:, 10:11], fin[:, 8:9], fin[:, 9:10], A.mult)
            nc.sync.dma_start(out_dram[it, :], fin[:, 10:11])

    nc.compile()
    return nc


def kernel(traj, veh_att, raster, mapixes, dx, _trace=False):
    _install_ntff_hook()
    from concourse.bass_utils import run_bass_kernel_spmd

    traj = np.ascontiguousarray(traj, np.float32)
    veh_att = np.ascontiguousarray(veh_att, np.float32)
    raster = np.ascontiguousarray(raster, np.float32)
    mapixes = np.ascontiguousarray(mapixes).astype(np.int64)
    dxf = np.float32(np.asarray(dx).reshape(-1)[0])

    # ---- host layout prep ----
    # Y8 pack: words[m, y8, x] in [0, 256)
    r8 = (raster >= 0.5).astype(np.uint16).reshape(N_MAPS, MAP_H // 8, 8, MAP_W)
    wts = (1 << np.arange(8)).astype(np.uint16)
    words = (r8 * wts[None, None, :, None]).sum(axis=2).astype(np.float16)  # [4,256,2048]

    # per-map ap_gather table [128, 16384, 2]: tab[p, x16*128+y8p, s] =
    #   words[m, (y8p*2+s... careful: e=(x16)*128+(iy>>4); d-pair = words y8=(iy>>3)
    # pair index y8p = iy//16; within pair s=(iy>>3)&1 -> y8 = y8p*2+s
    tabs = []
    for m in range(N_MAPS):
        wm = words[m]  # [256, 2048]
        t = np.zeros((128, 16384, 2), np.float16)
        j = (np.arange(128) % 16)
        x16 = np.arange(128)
        y8p = np.arange(128)
        # t[p, x16*128 + y8p, s] = wm[y8p*2+s, x16*16 + p%16]
        xx = (x16[:, None] * 16)[None, :, :] + j[:, None, None]      # [128p,128x16,1]->x
        for s in range(2):
            # index arrays: [128, 128x16, 128y8p]
            t[:, :, s].reshape(128, 128, 128)[:, :, :] = \
                wm[(y8p * 2 + s)[None, None, :], xx]
        tabs.append(t.reshape(128, 16384 * 2))

    mt = np.zeros((128, 32, 2), np.float16)
    mt[np.arange(128), 2 * (np.arange(128) % 16), :] = 1
    mt = mt.reshape(128, 64)
    ones8 = np.zeros((128, 8), np.float16)
    ones8[np.arange(128), np.arange(128) // 16] = 1

    uu2, vv2 = np.meshgrid(_UU10, _VV20, indexing="ij")
    uu_rep = np.broadcast_to(uu2.reshape(1, P), (128, P)).astype(np.float32).copy()
    vv_rep = np.broadcast_to(vv2.reshape(1, P), (128, P)).astype(np.float32).copy()
    iotaf = np.broadcast_to(np.arange(P, dtype=np.float32)[None, :], (128, P)).copy()
    dxrep = np.full((128, 1), dxf, np.float32)

    # ---- shard agents by map, 2 cores per map ----
    core_agents = [[] for _ in range(N_CORES)]
    for m in range(N_MAPS):
        ags = np.where(mapixes == m)[0]
        half = (len(ags) + 1) // 2
        core_agents[2 * m] = list(ags[:half])
        core_agents[2 * m + 1] = list(ags[half:])

    n_states = [len(a) * T for a in core_agents]
    n_tiles = max(1, int(np.ceil(max(n_states) / 128)))
    S = n_tiles * 128

    in_maps = []
    state_maps = []
    for c in range(N_CORES):
        ags = core_agents[c]
        tr = np.zeros((S, 4), np.float32)
        at = np.zeros((S, 2), np.float32)
        smap = np.full(S, -1, np.int64)
        if ags:
            idx = np.array([(a * T + t) for a in ags for t in range(T)])
            tr[:len(idx)] = traj.reshape(NA * T, 4)[idx]
            at[:len(idx)] = veh_att[np.repeat(ags, T)]
            smap[:len(idx)] = idx
        # pad rows: safe in-bounds values
        pad = smap < 0
        tr[pad] = np.array([100.0, 100.0, 1.0, 0.0], np.float32)
        at[pad] = np.array([4.0, 2.0], np.float32)
        in_maps.append({
            "tab": tabs[c // 2], "mt": mt, "ones8": ones8, "uu": uu_rep,
            "vv": vv_rep, "iotaf": iotaf, "dxrep": dxrep,
            "trajsh": tr, "attsh": at,
        })
        state_maps.append(smap)

    if n_tiles not in _PROGRAM_CACHE:
        _PROGRAM_CACHE[n_tiles] = _build_program(n_tiles)
    nc = _PROGRAM_CACHE[n_tiles]

    try:
        res = run_bass_kernel_spmd(nc, in_maps, list(range(N_CORES)), trace=_trace)
    except Exception:
        if not _trace:
            raise
        res = run_bass_kernel_spmd(nc, in_maps, list(range(N_CORES)), trace=False)
    kernel.last_results = res

    out = np.zeros(NA * T, np.float32)
    for c in range(N_CORES):
        o = res.results[c]["outsh"].reshape(-1)
        valid = state_maps[c] >= 0
        out[state_maps[c][valid]] = o[valid]
    return out



# revision 3
# speedup vs baseline: 1.0620x; 1.0620x over previous
"""Bass/TRN2 kernel for nn_EnvCollLoss (oriented-footprint raster collision loss).

Strategy: agents sharded by map index across 8 cores (2 cores per map).
Each core holds its map as a Y8-bitpacked fp16 ap_gather table in SBUF
(partition j holds raster columns x===j mod 16; index e=(ix//16)*128+(iy//16)
returns the two 8-row packed words covering rows [16*(iy//16), +16)).
Per point: gather word-pair + a one-hot pair keyed by (ix%16, s=(iy>>3)&1)
(the one-hot also selects which of the two 8-row words), mask-multiply, then
16 accumulating one-hot-stationary matmuls reduce the 16 candidate partitions
directly into state-major PSUM layout (no repartition DMAs). Penalty uses
dist = sqrt(min d2) over colliding points (rotation-invariant d2 from
constant uu^2/vv^2 tables), so no argmin index extraction is needed.
Per-state scalar math is hoisted into [128, n_tiles]-wide prologue/epilogue.
"""
import sys
import types
import numpy as np
from contextlib import ExitStack

NA, T = 256, 100
N_MAPS, MAP_H, MAP_W = 4, 2048, 2048
PU, PV = 10, 20
P = PU * PV  # 200
N_CORES = 8

# jnp.linspace(-0.5, 0.5, 10/20, dtype=float32) exact values (validated vs jax)
_UU10 = np.array([-0.5, -0.3888889, -0.2777778, -0.16666667, -0.05555556,
                  0.05555556, 0.16666667, 0.2777778, 0.3888889, 0.5], dtype=np.float32)
_VV20 = np.linspace(-0.5, 0.5, 20, dtype=np.float32)


def _install_ntff_hook():
    import antenv
    if "antenv.axon_hooks" in sys.modules:
        return
    try:
        from trn_agent_boot.trn_boot import _ntff_profile_via_ctypes
        hook = _ntff_profile_via_ctypes("/opt/axon/libaxon_pjrt.so")
    except Exception:
        hook = None
    mod = types.ModuleType("antenv.axon_hooks")
    mod._hook = hook
    mod.get_axon_ntff_profile_hook = lambda: mod._hook
    mod.set_axon_ntff_profile_hook = lambda h: setattr(mod, "_hook", h)
    sys.modules["antenv.axon_hooks"] = mod
    antenv.axon_hooks = mod


_PROGRAM_CACHE = {}


def _build_program(n_tiles):
    import concourse.tile as tile
    from concourse import bacc, mybir

    dt = mybir.dt
    A = mybir.AluOpType
    NT = n_tiles

    nc = bacc.Bacc("TRN2", target_bir_lowering=False, debug=False,
                   enable_asserts=False, num_devices=N_CORES)

    tab_in = nc.dram_tensor("tab", [128, 16384 * 2], dt.float16, kind="ExternalInput").ap()
    mt_in = nc.dram_tensor("mt2", [128, 64], dt.float16, kind="ExternalInput").ap()
    st_in = nc.dram_tensor("stat", [128, 16 * 128], dt.float16, kind="ExternalInput").ap()
    uu_in = nc.dram_tensor("uu", [128, P], dt.float32, kind="ExternalInput").ap()
    vv_in = nc.dram_tensor("vv", [128, P], dt.float32, kind="ExternalInput").ap()
    uq_in = nc.dram_tensor("uusq", [128, P], dt.float32, kind="ExternalInput").ap()
    vq_in = nc.dram_tensor("vvsq", [128, P], dt.float32, kind="ExternalInput").ap()
    dx_in = nc.dram_tensor("dxrep", [128, 1], dt.float32, kind="ExternalInput").ap()
    cx_in = nc.dram_tensor("cxs", [128, NT], dt.float32, kind="ExternalInput").ap()
    cy_in = nc.dram_tensor("cys", [128, NT], dt.float32, kind="ExternalInput").ap()
    hx_in = nc.dram_tensor("hxs", [128, NT], dt.float32, kind="ExternalInput").ap()
    hy_in = nc.dram_tensor("hys", [128, NT], dt.float32, kind="ExternalInput").ap()
    lL_in = nc.dram_tensor("Ls", [128, NT], dt.float32, kind="ExternalInput").ap()
    lW_in = nc.dram_tensor("Ws", [128, NT], dt.float32, kind="ExternalInput").ap()
    out_dram = nc.dram_tensor("outsh", [128, NT], dt.float32, kind="ExternalOutput").ap()

    with tile.TileContext(nc) as tc, ExitStack() as ctx:
        cpool = ctx.enter_context(tc.tile_pool(name="const", bufs=1))
        wpool = ctx.enter_context(tc.tile_pool(name="work", bufs=2))
        gpool = ctx.enter_context(tc.tile_pool(name="gath", bufs=2))
        ppool = ctx.enter_context(tc.tile_pool(name="ps", bufs=4, space="PSUM"))

        tab = cpool.tile([128, 16384 * 2], dt.float16)
        nc.sync.dma_start(tab[:], tab_in)
        mt2 = cpool.tile([128, 64], dt.float16)
        nc.sync.dma_start(mt2[:], mt_in)
        stat = cpool.tile([128, 16 * 128], dt.float16)
        nc.sync.dma_start(stat[:], st_in)
        uu = cpool.tile([128, P], dt.float32)
        nc.sync.dma_start(uu[:], uu_in)
        vv = cpool.tile([128, P], dt.float32)
        nc.sync.dma_start(vv[:], vv_in)
        uusq = cpool.tile([128, P], dt.float32)
        nc.sync.dma_start(uusq[:], uq_in)
        vvsq = cpool.tile([128, P], dt.float32)
        nc.sync.dma_start(vvsq[:], vq_in)
        dxrep = cpool.tile([128, 1], dt.float32)
        nc.sync.dma_start(dxrep[:], dx_in)
        cxT = cpool.tile([128, NT], dt.float32)
        nc.sync.dma_start(cxT[:], cx_in)
        cyT = cpool.tile([128, NT], dt.float32)
        nc.sync.dma_start(cyT[:], cy_in)
        hx0T = cpool.tile([128, NT], dt.float32)
        nc.sync.dma_start(hx0T[:], hx_in)
        hy0T = cpool.tile([128, NT], dt.float32)
        nc.sync.dma_start(hy0T[:], hy_in)
        LT = cpool.tile([128, NT], dt.float32)
        nc.sync.dma_start(LT[:], lL_in)
        WT = cpool.tile([128, NT], dt.float32)
        nc.sync.dma_start(WT[:], lW_in)

        invdx = cpool.tile([128, 1], dt.float32)
        nc.vector.reciprocal(invdx[:], dxrep[:])

        # ---- prologue: per-state scalars, [128, NT]-wide ----
        # heading normalization via the same near-1 closed form as jax norm
        pg = cpool.tile([128, NT * 12], dt.float32)
        pv = pg[:].rearrange("p (c t) -> p c t", c=12)
        t0, t1, x2, delta = pv[:, 0, :], pv[:, 1, :], pv[:, 2, :], pv[:, 3, :]
        nc.vector.tensor_tensor(t0, hx0T[:], hx0T[:], A.mult)
        nc.vector.tensor_tensor(t1, hy0T[:], hy0T[:], A.mult)
        nc.vector.tensor_tensor(x2, t0, t1, A.add)
        nc.vector.tensor_scalar(delta, x2, -1.0, None, A.add)
        d5, dsq, den = pv[:, 4, :], pv[:, 5, :], pv[:, 6, :]
        nc.vector.tensor_scalar(d5, delta, 0.5, None, A.mult)
        nc.vector.tensor_tensor(dsq, delta, delta, A.mult)
        nc.vector.scalar_tensor_tensor(den, dsq, -0.125, d5, A.mult, A.add)
        nc.vector.tensor_scalar(den, den, 1.0, None, A.add)
        invn = cpool.tile([128, NT], dt.float32)
        nc.vector.reciprocal(invn[:], den)
        hxT = cpool.tile([128, NT], dt.float32)
        nc.vector.tensor_tensor(hxT[:], hx0T[:], invn[:], A.mult)
        hyT = cpool.tile([128, NT], dt.float32)
        nc.vector.tensor_tensor(hyT[:], hy0T[:], invn[:], A.mult)
        nhyT = cpool.tile([128, NT], dt.float32)
        nc.vector.tensor_scalar(nhyT[:], hyT[:], -1.0, None, A.mult)
        bxT = cpool.tile([128, NT], dt.float32)
        nc.vector.tensor_scalar(bxT[:], cxT[:], invdx[:], None, A.mult)
        byT = cpool.tile([128, NT], dt.float32)
        nc.vector.tensor_scalar(byT[:], cyT[:], invdx[:], None, A.mult)
        # -L^2, -W^2 for the d2-from-constants trick; pen = sqrt(L^2/4+W^2/4)
        tL, tW = pv[:, 7, :], pv[:, 8, :]
        nc.vector.tensor_tensor(tL, LT[:], LT[:], A.mult)
        nc.vector.tensor_tensor(tW, WT[:], WT[:], A.mult)
        nL2T = cpool.tile([128, NT], dt.float32)
        nc.vector.tensor_scalar(nL2T[:], tL, -1.0, None, A.mult)
        nW2T = cpool.tile([128, NT], dt.float32)
        nc.vector.tensor_scalar(nW2T[:], tW, -1.0, None, A.mult)
        p2, s0, rr = pv[:, 9, :], pv[:, 10, :], pv[:, 11, :]
        nc.vector.tensor_tensor(p2, tL, tW, A.add)
        nc.vector.tensor_scalar(p2, p2, 0.25, None, A.mult)
        nc.scalar.activation(s0, p2, mybir.ActivationFunctionType.Sqrt)
        nc.vector.reciprocal(rr, s0)
        nc.vector.tensor_tensor(rr, p2, rr, A.mult)
        nc.vector.tensor_tensor(rr, rr, s0, A.add)
        invpenT = cpool.tile([128, NT], dt.float32)
        nc.vector.tensor_scalar(invpenT[:], rr, 0.5, None, A.mult)   # pen
        nc.vector.reciprocal(invpenT[:], invpenT[:])                 # 1/pen

        res = cpool.tile([128, NT], dt.float32)

        for it in range(n_tiles):
            Lc, Wc = LT[:, it:it + 1], WT[:, it:it + 1]
            hxc, hyc, nhyc = hxT[:, it:it + 1], hyT[:, it:it + 1], nhyT[:, it:it + 1]
            bxc, byc = bxT[:, it:it + 1], byT[:, it:it + 1]

            bu = wpool.tile([128, P], dt.float32, tag="bu")
            nc.vector.tensor_scalar(bu[:], uu[:], Lc, None, A.mult)
            bv = wpool.tile([128, P], dt.float32, tag="bv")
            nc.vector.tensor_scalar(bv[:], vv[:], Wc, None, A.mult)
            t1 = wpool.tile([128, P], dt.float32, tag="t1")
            nc.vector.tensor_scalar(t1[:], bu[:], hxc, None, A.mult)
            ox = wpool.tile([128, P], dt.float32, tag="ox")
            nc.vector.scalar_tensor_tensor(ox[:], bv[:], nhyc, t1[:], A.mult, A.add)
            nc.vector.tensor_scalar(t1[:], bu[:], hyc, None, A.mult)
            oy = wpool.tile([128, P], dt.float32, tag="oy")
            nc.vector.scalar_tensor_tensor(oy[:], bv[:], hxc, t1[:], A.mult, A.add)

            # exact floor((ctr + o)/dx): pw = o*invdx + ctr*invdx (exact, dx=2^-k)
            ixi = wpool.tile([128, P], dt.int32, tag="ixi")
            iyi = wpool.tile([128, P], dt.int32, tag="iyi")
            for (ov, bc, resI) in ((ox, bxc, ixi), (oy, byc, iyi)):
                pw = wpool.tile([128, P], dt.float32, tag="pw")
                nc.vector.tensor_scalar(pw[:], ov[:], invdx[:], bc, A.mult, A.add)
                ci = wpool.tile([128, P], dt.int32, tag="ci")
                nc.vector.tensor_copy(ci[:], pw[:])                          # RNE
                cf = wpool.tile([128, P], dt.float32, tag="cf")
                nc.vector.tensor_copy(cf[:], ci[:])
                ad = wpool.tile([128, P], dt.float32, tag="ad")
                nc.vector.tensor_tensor(ad[:], cf[:], pw[:], A.is_gt)
                adi = wpool.tile([128, P], dt.int32, tag="adi")
                nc.vector.tensor_copy(adi[:], ad[:])
                nc.vector.tensor_tensor(resI[:], ci[:], adi[:], A.subtract)

            x16 = wpool.tile([128, P], dt.int32, tag="x16")
            nc.vector.tensor_scalar(x16[:], ixi[:], 4, None, A.logical_shift_right)
            y8p = wpool.tile([128, P], dt.int32, tag="y8p")
            nc.vector.tensor_scalar(y8p[:], iyi[:], 4, None, A.logical_shift_right)
            e32 = wpool.tile([128, P], dt.int32, tag="e32")
            nc.vector.scalar_tensor_tensor(e32[:], x16[:], 128, y8p[:], A.mult, A.add)
            e16 = wpool.tile([128, P], dt.int16, tag="e16")
            nc.vector.tensor_copy(e16[:], e32[:])
            jm2 = wpool.tile([128, P], dt.int32, tag="jm2")
            nc.vector.tensor_scalar(jm2[:], ixi[:], 15, 1, A.bitwise_and, A.logical_shift_left)
            sb = wpool.tile([128, P], dt.int32, tag="sb")
            nc.vector.tensor_scalar(sb[:], iyi[:], 3, 1, A.logical_shift_right, A.bitwise_and)
            j32 = wpool.tile([128, P], dt.int32, tag="j32")
            nc.vector.tensor_tensor(j32[:], jm2[:], sb[:], A.add)
            j16 = wpool.tile([128, P], dt.int16, tag="j16")
            nc.vector.tensor_copy(j16[:], j32[:])
            rbit = wpool.tile([128, P], dt.int32, tag="rbit")
            nc.vector.tensor_scalar(rbit[:], iyi[:], 7, None, A.bitwise_and)

            g1 = gpool.tile([128, 16 * P * 2], dt.float16, tag="g1")
            nc.gpsimd.ap_gather(g1[:], tab[:], e16[:], channels=128,
                                num_elems=16384, d=2, num_idxs=16 * P)
            g2 = gpool.tile([128, 16 * P * 2], dt.float16, tag="g2")
            nc.gpsimd.ap_gather(g2[:], mt2[:], j16[:], channels=128,
                                num_elems=32, d=2, num_idxs=16 * P)
            nc.vector.tensor_tensor(g1[:], g1[:], g2[:], A.mult)

            # 16 accumulating matmuls: psum[16g+k, (i b)] = sum_{p in g} g1[p, i,k,b]
            pt = ppool.tile([128, P * 2], dt.float32, tag="pt")
            mv4 = g1[:].rearrange("p (i k b) -> p k i b", i=P, k=16, b=2)
            st3 = stat[:].rearrange("p (k o) -> p k o", k=16)
            for k in range(16):
                nc.tensor.matmul(pt[:], st3[:, k, :], mv4[:, k, :, :],
                                 start=(k == 0), stop=(k == 15))

            # pair-add the two s-slots (one is zero); only one PSUM operand
            # per instruction is allowed, so stage slot0 via the scalar engine
            ptv = pt[:].rearrange("p (i b) -> p i b", b=2)
            w0 = wpool.tile([128, P], dt.float32, tag="w0")
            nc.scalar.copy(w0[:], ptv[:, :, 0:1].rearrange("p i b -> p (i b)"))
            wred = wpool.tile([128, P], dt.float32, tag="wred")
            nc.vector.tensor_tensor(
                wred[:], w0[:],
                ptv[:, :, 1:2].rearrange("p i b -> p (i b)"), A.add)
            wi = wpool.tile([128, P], dt.int32, tag="wi")
            nc.vector.tensor_copy(wi[:], wred[:])
            nc.vector.tensor_tensor(wi[:], wi[:], rbit[:], A.logical_shift_right)
            nc.vector.tensor_scalar(wi[:], wi[:], 1, 1, A.bitwise_and, A.bitwise_xor)
            cbf = wpool.tile([128, P], dt.float32, tag="cbf")
            nc.vector.tensor_copy(cbf[:], wi[:])

            # key = (16 - d2) * coll,  d2 = L^2 uu^2 + W^2 vv^2 (exact enough)
            tw = wpool.tile([128, P], dt.float32, tag="tw")
            nc.vector.tensor_scalar(tw[:], vvsq[:], nW2T[:, it:it + 1], 16.0, A.mult, A.add)
            ckey = wpool.tile([128, P], dt.float32, tag="ckey")
            nc.vector.scalar_tensor_tensor(ckey[:], uusq[:], nL2T[:, it:it + 1], tw[:],
                                           A.mult, A.add)
            nc.vector.tensor_tensor(ckey[:], ckey[:], cbf[:], A.mult)
            mx8 = wpool.tile([128, 8], dt.float32, tag="mx8")
            nc.vector.max(mx8[:], ckey[:])
            nc.vector.tensor_copy(res[:, it:it + 1], mx8[:, 0:1])

        # ---- epilogue: penalty from max-key, [128, NT]-wide ----
        eg = cpool.tile([128, NT * 4], dt.float32)
        ev = eg[:].rearrange("p (c t) -> p c t", c=4)
        d2m, es0, er, val = ev[:, 0, :], ev[:, 1, :], ev[:, 2, :], ev[:, 3, :]
        nc.vector.tensor_scalar(d2m, res[:], -1.0, 16.0, A.mult, A.add)
        nc.scalar.activation(es0, d2m, mybir.ActivationFunctionType.Sqrt)
        nc.vector.reciprocal(er, es0)
        nc.vector.tensor_tensor(er, d2m, er, A.mult)
        nc.vector.tensor_tensor(er, er, es0, A.add)
        nc.vector.tensor_scalar(er, er, 0.5, None, A.mult)       # dist
        nc.vector.tensor_tensor(er, er, invpenT[:], A.mult)
        nc.vector.tensor_scalar(er, er, -1.0, 1.0, A.mult, A.add)  # 1 - dist/pen
        nc.vector.tensor_scalar(val, res[:], 0.0, None, A.is_gt)
        out_t = cpool.tile([128, NT], dt.float32)
        nc.vector.tensor_tensor(out_t[:], er, val, A.mult)
        nc.sync.dma_start(out_dram, out_t[:])

    nc.compile()
    return nc


def kernel(traj, veh_att, raster, mapixes, dx, _trace=False):
    _install_ntff_hook()
    from concourse.bass_utils import run_bass_kernel_spmd

    traj = np.ascontiguousarray(traj, np.float32)
    veh_att = np.ascontiguousarray(veh_att, np.float32)
    raster = np.ascontiguousarray(raster, np.float32)
    mapixes = np.ascontiguousarray(mapixes).astype(np.int64)
    dxf = np.float32(np.asarray(dx).reshape(-1)[0])

    # ---- host layout prep ----
    # Y8 pack: words[m, y8, x] in [0, 256)
    r8 = (raster >= 0.5).astype(np.uint16).reshape(N_MAPS, MAP_H // 8, 8, MAP_W)
    wts = (1 << np.arange(8)).astype(np.uint16)
    words = (r8 * wts[None, None, :, None]).sum(axis=2).astype(np.float16)  # [4,256,2048]

    # per-map ap_gather table [128, 16384, 2]: partition p holds columns
    # x = 16*x16 + (p%16); entry e = x16*128 + y8p is words y8 = 2*y8p + {0,1}
    tabs = []
    for m in range(N_MAPS):
        wm = words[m]  # [256, 2048]
        t = np.zeros((128, 16384, 2), np.float16)
        j = (np.arange(128) % 16)
        x16 = np.arange(128)
        y8p = np.arange(128)
        xx = (x16[:, None] * 16)[None, :, :] + j[:, None, None]
        for s in range(2):
            t[:, :, s].reshape(128, 128, 128)[:, :, :] = \
                wm[(y8p * 2 + s)[None, None, :], xx]
        tabs.append(t.reshape(128, 16384 * 2))

    # one-hot (j, s) table: mt2[p, 2j+s, b] = (j == p%16) & (b == s)
    mt2 = np.zeros((128, 32, 2), np.float16)
    pj = np.arange(128) % 16
    mt2[np.arange(128), 2 * pj, 0] = 1
    mt2[np.arange(128), 2 * pj + 1, 1] = 1
    mt2 = mt2.reshape(128, 64)

    # 16 one-hot stationaries: stat[p, 128k + 16*(p//16)+k] = 1
    stat = np.zeros((128, 16, 128), np.float16)
    pp = np.arange(128)
    for k in range(16):
        stat[pp, k, 16 * (pp // 16) + k] = 1
    stat = stat.reshape(128, 16 * 128)

    uu2, vv2 = np.meshgrid(_UU10, _VV20, indexing="ij")
    uu_rep = np.broadcast_to(uu2.reshape(1, P), (128, P)).astype(np.float32).copy()
    vv_rep = np.broadcast_to(vv2.reshape(1, P), (128, P)).astype(np.float32).copy()
    uusq = (uu_rep * uu_rep).astype(np.float32)
    vvsq = (vv_rep * vv_rep).astype(np.float32)
    dxrep = np.full((128, 1), dxf, np.float32)

    # ---- shard agents by map, 2 cores per map ----
    core_agents = [[] for _ in range(N_CORES)]
    for m in range(N_MAPS):
        ags = np.where(mapixes == m)[0]
        half = (len(ags) + 1) // 2
        core_agents[2 * m] = list(ags[:half])
        core_agents[2 * m + 1] = list(ags[half:])

    n_states = [len(a) * T for a in core_agents]
    n_tiles = max(1, int(np.ceil(max(n_states) / 128)))
    S = n_tiles * 128

    traj_flat = traj.reshape(NA * T, 4)
    in_maps = []
    state_maps = []
    for c in range(N_CORES):
        ags = core_agents[c]
        tr = np.zeros((S, 4), np.float32)
        at = np.zeros((S, 2), np.float32)
        smap = np.full(S, -1, np.int64)
        if ags:
            idx = np.array([(a * T + t) for a in ags for t in range(T)])
            tr[:len(idx)] = traj_flat[idx]
            at[:len(idx)] = veh_att[np.repeat(ags, T)]
            smap[:len(idx)] = idx
        pad = smap < 0
        tr[pad] = np.array([100.0, 100.0, 1.0, 0.0], np.float32)
        at[pad] = np.array([4.0, 2.0], np.float32)
        # [128, NT] layouts: state t = it*128 + p  ->  [p, it]
        trt = tr.reshape(n_tiles, 128, 4).transpose(1, 0, 2)
        att2 = at.reshape(n_tiles, 128, 2).transpose(1, 0, 2)
        in_maps.append({
            "tab": tabs[c // 2], "mt2": mt2, "stat": stat,
            "uu": uu_rep, "vv": vv_rep, "uusq": uusq, "vvsq": vvsq,
            "dxrep": dxrep,
            "cxs": np.ascontiguousarray(trt[:, :, 0]),
            "cys": np.ascontiguousarray(trt[:, :, 1]),
            "hxs": np.ascontiguousarray(trt[:, :, 2]),
            "hys": np.ascontiguousarray(trt[:, :, 3]),
            "Ls": np.ascontiguousarray(att2[:, :, 0]),
            "Ws": np.ascontiguousarray(att2[:, :, 1]),
        })
        state_maps.append(smap)

    if n_tiles not in _PROGRAM_CACHE:
        _PROGRAM_CACHE[n_tiles] = _build_program(n_tiles)
    nc = _PROGRAM_CACHE[n_tiles]

    try:
        res = run_bass_kernel_spmd(nc, in_maps, list(range(N_CORES)), trace=_trace)
    except Exception:
        if not _trace:
            raise
        res = run_bass_kernel_spmd(nc, in_maps, list(range(N_CORES)), trace=False)
    kernel.last_results = res

    out = np.zeros(NA * T, np.float32)
    for c in range(N_CORES):
        o = res.results[c]["outsh"].T.reshape(-1)   # [128, NT] -> state order
        valid = state_maps[c] >= 0
        out[state_maps[c][valid]] = o[valid]
    return out


# revision 15
# speedup vs baseline: 5.3846x; 5.0703x over previous
"""Bass/TRN2 kernel for nn_EnvCollLoss (oriented-footprint raster collision loss).

v3: patch-gather design. Agents sharded by map (2 cores/map). Per state we
fetch a 28-col x 48-row window of the raster with SEVEN pooled ap_gather
indices (4-column groups x two 24-row fp32-packed words, d=8) instead of 200
per-point indices -- ap_gather cost is ~27ns/index, so this cuts the gather
from ~175us to ~11us per tile. A one-hot (by group-partition) gather + 16
accumulating one-hot-stationary matmuls reduce the 16 candidate partitions
into state-major PSUM. Each column's two 24-row words are fused into one
int32 covering rows [y0, y0+32); a 3-pass dense select over [128, 200x28]
(is_equal against an iota via stride-0 broadcast APs, mask-mult, or-reduce)
picks each point's column word, then a per-point shift extracts the bit.
Penalty uses dist = sqrt(min d2) over colliding points with d2 from constant
uu^2/vv^2 tables; per-state scalar math is hoisted into [128, n_tiles]-wide
prologue/epilogue. Stage-B work of tile N-1 is emitted between tile N's
gathers and matmuls so vector work hides gather/PE latency.
"""
import sys
import types
import numpy as np
from contextlib import ExitStack

NA, T = 256, 100
N_MAPS, MAP_H, MAP_W = 4, 2048, 2048
PU, PV = 10, 20
P = PU * PV  # 200
N_CORES = 8
NG = 7            # 4-col groups per state window (28 cols)
NC = 4 * NG       # 28 columns
QB = 86           # 24-row blocks per column (rows < 2064)

# jnp.linspace(-0.5, 0.5, 10/20, dtype=float32) exact values (validated vs jax)
_UU10 = np.array([-0.5, -0.3888889, -0.2777778, -0.16666667, -0.05555556,
                  0.05555556, 0.16666667, 0.2777778, 0.3888889, 0.5], dtype=np.float32)
_VV20 = np.linspace(-0.5, 0.5, 20, dtype=np.float32)


def _install_ntff_hook():
    import antenv
    if "antenv.axon_hooks" in sys.modules:
        return
    try:
        from trn_agent_boot.trn_boot import _ntff_profile_via_ctypes
        hook = _ntff_profile_via_ctypes("/opt/axon/libaxon_pjrt.so")
    except Exception:
        hook = None
    mod = types.ModuleType("antenv.axon_hooks")
    mod._hook = hook
    mod.get_axon_ntff_profile_hook = lambda: mod._hook
    mod.set_axon_ntff_profile_hook = lambda h: setattr(mod, "_hook", h)
    sys.modules["antenv.axon_hooks"] = mod
    antenv.axon_hooks = mod


_PROGRAM_CACHE = {}


def _build_program(n_tiles):
    import concourse.tile as tile
    from concourse import bacc, mybir
    from concourse.bass import broadcast_tensor_aps

    dt = mybir.dt
    A = mybir.AluOpType
    NT = n_tiles
    INV24 = float(np.float32(1.0) / np.float32(24.0))

    nc = bacc.Bacc("TRN2", target_bir_lowering=False, debug=False,
                   enable_asserts=False, num_devices=N_CORES)

    tab_in = nc.dram_tensor("tab", [128, 2752 * 8], dt.float32, kind="ExternalInput").ap()
    mt_in = nc.dram_tensor("mt2", [128, 16 * 8], dt.float32, kind="ExternalInput").ap()
    st_in = nc.dram_tensor("stat", [128, 16 * 128], dt.float32, kind="ExternalInput").ap()
    uu_in = nc.dram_tensor("uu", [128, P], dt.float32, kind="ExternalInput").ap()
    vv_in = nc.dram_tensor("vv", [128, P], dt.float32, kind="ExternalInput").ap()
    uq_in = nc.dram_tensor("uusq", [128, P], dt.float32, kind="ExternalInput").ap()
    vq_in = nc.dram_tensor("vvsq", [128, P], dt.float32, kind="ExternalInput").ap()
    i7_in = nc.dram_tensor("iota7", [128, NG], dt.float32, kind="ExternalInput").ap()
    i28_in = nc.dram_tensor("iota28", [128, NC], dt.int32, kind="ExternalInput").ap()
    dx_in = nc.dram_tensor("dxrep", [128, 1], dt.float32, kind="ExternalInput").ap()
    cx_in = nc.dram_tensor("cxs", [128, NT], dt.float32, kind="ExternalInput").ap()
    cy_in = nc.dram_tensor("cys", [128, NT], dt.float32, kind="ExternalInput").ap()
    hx_in = nc.dram_tensor("hxs", [128, NT], dt.float32, kind="ExternalInput").ap()
    hy_in = nc.dram_tensor("hys", [128, NT], dt.float32, kind="ExternalInput").ap()
    lL_in = nc.dram_tensor("Ls", [128, NT], dt.float32, kind="ExternalInput").ap()
    lW_in = nc.dram_tensor("Ws", [128, NT], dt.float32, kind="ExternalInput").ap()
    out_dram = nc.dram_tensor("outsh", [128, NT], dt.float32, kind="ExternalOutput").ap()

    with tile.TileContext(nc) as tc, ExitStack() as ctx:
        cpool = ctx.enter_context(tc.tile_pool(name="const", bufs=1))
        wpool = ctx.enter_context(tc.tile_pool(name="work", bufs=2))
        mpool = ctx.enter_context(tc.tile_pool(name="msel", bufs=1))
        gpool = ctx.enter_context(tc.tile_pool(name="gath", bufs=2))
        ppool = ctx.enter_context(tc.tile_pool(name="ps", bufs=2, space="PSUM"))

        def cload(name, shape, dtp, src):
            t = cpool.tile(shape, dtp, tag=name)
            nc.sync.dma_start(t[:], src)
            return t

        tab = cload("tab", [128, 2752 * 8], dt.float32, tab_in)
        mt2 = cload("mt2", [128, 16 * 8], dt.float32, mt_in)
        stat = cload("stat", [128, 16 * 128], dt.float32, st_in)
        uu = cload("uu", [128, P], dt.float32, uu_in)
        vv = cload("vv", [128, P], dt.float32, vv_in)
        uusq = cload("uusq", [128, P], dt.float32, uq_in)
        vvsq = cload("vvsq", [128, P], dt.float32, vq_in)
        iota7 = cload("iota7", [128, NG], dt.float32, i7_in)
        iota28 = cload("iota28", [128, NC], dt.int32, i28_in)
        dxrep = cload("dxrep", [128, 1], dt.float32, dx_in)
        cxT = cload("cxT", [128, NT], dt.float32, cx_in)
        cyT = cload("cyT", [128, NT], dt.float32, cy_in)
        hx0T = cload("hx0T", [128, NT], dt.float32, hx_in)
        hy0T = cload("hy0T", [128, NT], dt.float32, hy_in)
        LT = cload("LT", [128, NT], dt.float32, lL_in)
        WT = cload("WT", [128, NT], dt.float32, lW_in)

        invdx = cpool.tile([128, 1], dt.float32)
        nc.vector.reciprocal(invdx[:], dxrep[:])

        # ---- prologue: per-state scalars, [128, NT]-wide ----
        pg = cpool.tile([128, NT * 16], dt.float32)
        pv = pg[:].rearrange("p (c t) -> p c t", c=16)
        t0, t1, x2, delta = pv[:, 0, :], pv[:, 1, :], pv[:, 2, :], pv[:, 3, :]
        nc.vector.tensor_tensor(t0, hx0T[:], hx0T[:], A.mult)
        nc.vector.tensor_tensor(t1, hy0T[:], hy0T[:], A.mult)
        nc.vector.tensor_tensor(x2, t0, t1, A.add)
        nc.vector.tensor_scalar(delta, x2, -1.0, None, A.add)
        d5, dsq, den = pv[:, 4, :], pv[:, 5, :], pv[:, 6, :]
        nc.vector.tensor_scalar(d5, delta, 0.5, None, A.mult)
        nc.vector.tensor_tensor(dsq, delta, delta, A.mult)
        nc.vector.scalar_tensor_tensor(den, dsq, -0.125, d5, A.mult, A.add)
        nc.vector.tensor_scalar(den, den, 1.0, None, A.add)
        invn = cpool.tile([128, NT], dt.float32)
        nc.vector.reciprocal(invn[:], den)
        hxT = cpool.tile([128, NT], dt.float32)
        nc.vector.tensor_tensor(hxT[:], hx0T[:], invn[:], A.mult)
        hyT = cpool.tile([128, NT], dt.float32)
        nc.vector.tensor_tensor(hyT[:], hy0T[:], invn[:], A.mult)
        nhyT = cpool.tile([128, NT], dt.float32)
        nc.vector.tensor_scalar(nhyT[:], hyT[:], -1.0, None, A.mult)
        bxT = cpool.tile([128, NT], dt.float32)
        nc.vector.tensor_scalar(bxT[:], cxT[:], invdx[:], None, A.mult)
        byT = cpool.tile([128, NT], dt.float32)
        nc.vector.tensor_scalar(byT[:], cyT[:], invdx[:], None, A.mult)
        # -L^2, -W^2; pen = sqrt(L^2/4 + W^2/4); 1/pen
        tL, tW = pv[:, 7, :], pv[:, 8, :]
        nc.vector.tensor_tensor(tL, LT[:], LT[:], A.mult)
        nc.vector.tensor_tensor(tW, WT[:], WT[:], A.mult)
        nL2T = cpool.tile([128, NT], dt.float32)
        nc.vector.tensor_scalar(nL2T[:], tL, -1.0, None, A.mult)
        nW2T = cpool.tile([128, NT], dt.float32)
        nc.vector.tensor_scalar(nW2T[:], tW, -1.0, None, A.mult)
        p2, s0, rr = pv[:, 9, :], pv[:, 10, :], pv[:, 11, :]
        nc.vector.tensor_tensor(p2, tL, tW, A.add)
        nc.vector.tensor_scalar(p2, p2, 0.25, None, A.mult)
        nc.scalar.activation(s0, p2, mybir.ActivationFunctionType.Sqrt)
        nc.vector.reciprocal(rr, s0)
        nc.vector.tensor_tensor(rr, p2, rr, A.mult)
        nc.vector.tensor_tensor(rr, rr, s0, A.add)
        invpenT = cpool.tile([128, NT], dt.float32)
        nc.vector.tensor_scalar(invpenT[:], rr, 0.5, None, A.mult)
        nc.vector.reciprocal(invpenT[:], invpenT[:])

        # window bases: cxp=floor(cx/dx); x0=4*floor((cxp-12)/4); G0=(x0/4);
        # y0=floor(cy/dx)-12; q0=floor(y0/24); sh0=y0-24*q0; sh1=24-sh0
        fci = cpool.tile([128, NT], dt.int32, tag="fci")

        def _floor(dst, src):
            # exact floor via RNE-to-int then adjust (src >= 0 here)
            fr = pv[:, 15, :]
            nc.vector.tensor_copy(fci[:], src)
            nc.vector.tensor_copy(fr, fci[:])
            ad = pv[:, 9, :]
            nc.vector.tensor_tensor(ad, fr, src, A.is_gt)
            nc.vector.tensor_tensor(dst, fr, ad, A.subtract)

        cxpf, cypf = pv[:, 10, :], pv[:, 11, :]
        _floor(cxpf, bxT[:])
        _floor(cypf, byT[:])
        G0fT = cpool.tile([128, NT], dt.float32)
        nc.vector.tensor_scalar(G0fT[:], cxpf, -12.0, 0.25, A.add, A.mult)
        _floor(G0fT[:], G0fT[:])
        x0fT = cpool.tile([128, NT], dt.float32)
        nc.vector.tensor_scalar(x0fT[:], G0fT[:], 4.0, None, A.mult)
        y0fT = cpool.tile([128, NT], dt.float32)
        nc.vector.tensor_scalar(y0fT[:], cypf, -12.0, None, A.add)
        q0fT = cpool.tile([128, NT], dt.float32)
        nc.vector.tensor_scalar(q0fT[:], y0fT[:], INV24, None, A.mult)
        _floor(q0fT[:], q0fT[:])
        sh0fT = cpool.tile([128, NT], dt.float32)
        nc.vector.scalar_tensor_tensor(sh0fT[:], q0fT[:], -24.0, y0fT[:], A.mult, A.add)
        sh1fT = cpool.tile([128, NT], dt.float32)
        nc.vector.tensor_scalar(sh1fT[:], sh0fT[:], -1.0, 24.0, A.mult, A.add)
        q0mfT = cpool.tile([128, NT], dt.float32)        # q0 - 172 (e-base rebase)
        nc.vector.tensor_scalar(q0mfT[:], q0fT[:], -172.0, None, A.add)

        res = cpool.tile([128, NT], dt.float32)
        ones28 = cpool.tile([128, NC], dt.float32)
        nc.vector.memset(ones28[:], 1.0)
        st3 = stat[:].rearrange("p (k o) -> p k o", k=16)

        prev = None  # stage-B state of previous tile

        def stage_b(pb):
            it, pt, dc, sh = pb
            # fuse each column's two 24-row words into one int32 window
            ptv = pt[:].rearrange("p (i c s) -> p (i c) s", i=NG, c=4, s=2)
            i0 = wpool.tile([128, NC], dt.int32, tag="i0")
            nc.vector.tensor_copy(i0[:], ptv[:, :, 0:1].rearrange("p q s -> p (q s)"))
            i1 = wpool.tile([128, NC], dt.int32, tag="i1")
            nc.vector.tensor_copy(i1[:], ptv[:, :, 1:2].rearrange("p q s -> p (q s)"))
            shbf = wpool.tile([128, NC], dt.float32, tag="shbf")
            shb = wpool.tile([128, NC], dt.int32, tag="shb")
            nc.vector.tensor_scalar(shbf[:], ones28[:], sh0fT[:, it:it + 1], None, A.mult)
            nc.vector.tensor_copy(shb[:], shbf[:])
            nc.vector.tensor_tensor(i0[:], i0[:], shb[:], A.logical_shift_right)
            nc.vector.tensor_scalar(shbf[:], ones28[:], sh1fT[:, it:it + 1], None, A.mult)
            nc.vector.tensor_copy(shb[:], shbf[:])
            nc.vector.tensor_tensor(i1[:], i1[:], shb[:], A.logical_shift_left)
            w32 = wpool.tile([128, NC], dt.int32, tag="w32")
            nc.vector.tensor_tensor(w32[:], i0[:], i1[:], A.bitwise_or)

            # dense column select: M = (dc == iota28); R = or_c(M * w32)
            M = mpool.tile([128, P * NC], dt.int32, tag="M")
            M3 = M[:].rearrange("p (t c) -> p t c", c=NC)
            dc3 = dc[:].rearrange("p (t o) -> p t o", o=1)
            io3 = iota28[:].rearrange("p (o c) -> p o c", o=1)
            dcB, ioB = broadcast_tensor_aps(dc3, io3)
            # full-width mask: (dc != c) - 1 = 0xFFFFFFFF where dc == c
            # (int32 `mult` routes through fp32 and mangles 32-bit words)
            nc.vector.tensor_tensor(M3, dcB, ioB, A.not_equal)
            nc.vector.tensor_scalar(M[:], M[:], 1, None, A.subtract)
            w3 = w32[:].rearrange("p (o c) -> p o c", o=1)
            w3B, M3B = broadcast_tensor_aps(w3, M3)
            nc.vector.tensor_tensor(M3, M3B, w3B, A.bitwise_and)
            R = wpool.tile([128, P], dt.int32, tag="R")
            nc.vector.tensor_reduce(R[:], M3, mybir.AxisListType.X, A.bitwise_or)

            # bit extract + key + reduce
            nc.vector.tensor_tensor(R[:], R[:], sh[:], A.logical_shift_right)
            nc.vector.tensor_scalar(R[:], R[:], 1, 1, A.bitwise_and, A.bitwise_xor)
            cbf = wpool.tile([128, P], dt.float32, tag="cbf")
            nc.vector.tensor_copy(cbf[:], R[:])
            tw = wpool.tile([128, P], dt.float32, tag="tw")
            nc.vector.tensor_scalar(tw[:], vvsq[:], nW2T[:, it:it + 1], 16.0,
                                    A.mult, A.add)
            ckey = wpool.tile([128, P], dt.float32, tag="ckey")
            nc.vector.scalar_tensor_tensor(ckey[:], uusq[:], nL2T[:, it:it + 1],
                                           tw[:], A.mult, A.add)
            nc.vector.tensor_tensor(ckey[:], ckey[:], cbf[:], A.mult)
            mx8 = wpool.tile([128, 8], dt.float32, tag="mx8")
            nc.vector.max(mx8[:], ckey[:])
            nc.vector.tensor_copy(res[:, it:it + 1], mx8[:, 0:1])

        for it in range(n_tiles):
            Lc, Wc = LT[:, it:it + 1], WT[:, it:it + 1]
            hxc, hyc, nhyc = hxT[:, it:it + 1], hyT[:, it:it + 1], nhyT[:, it:it + 1]

            # ---- stage A: per-point pixel math + window indices ----
            bu = wpool.tile([128, P], dt.float32, tag="bu")
            nc.scalar.activation(bu[:], uu[:], mybir.ActivationFunctionType.Copy,
                                 bias=0.0, scale=Lc)
            bv = wpool.tile([128, P], dt.float32, tag="bv")
            nc.scalar.activation(bv[:], vv[:], mybir.ActivationFunctionType.Copy,
                                 bias=0.0, scale=Wc)
            t1t = wpool.tile([128, P], dt.float32, tag="t1t")
            nc.scalar.activation(t1t[:], bu[:], mybir.ActivationFunctionType.Copy,
                                 bias=0.0, scale=hxc)
            t2t = wpool.tile([128, P], dt.float32, tag="t2t")
            nc.scalar.activation(t2t[:], bu[:], mybir.ActivationFunctionType.Copy,
                                 bias=0.0, scale=hyc)
            ox = wpool.tile([128, P], dt.float32, tag="ox")
            nc.vector.scalar_tensor_tensor(ox[:], bv[:], nhyc, t1t[:], A.mult, A.add)
            oy = wpool.tile([128, P], dt.float32, tag="oy")
            nc.vector.scalar_tensor_tensor(oy[:], bv[:], hxc, t2t[:], A.mult, A.add)

            dc = wpool.tile([128, P], dt.int32, tag="dc")
            sh = wpool.tile([128, P], dt.int32, tag="sh")
            for (ov, bc, b0T, resI) in ((ox, bxT, x0fT, dc), (oy, byT, y0fT, sh)):
                pw = wpool.tile([128, P], dt.float32, tag="pw")
                nc.vector.tensor_scalar(pw[:], ov[:], invdx[:], bc[:, it:it + 1],
                                        A.mult, A.add)
                ci = wpool.tile([128, P], dt.int32, tag="ci")
                nc.vector.tensor_copy(ci[:], pw[:])
                cf = wpool.tile([128, P], dt.float32, tag="cf")
                nc.vector.tensor_copy(cf[:], ci[:])
                ad = wpool.tile([128, P], dt.float32, tag="ad")
                nc.vector.tensor_tensor(ad[:], cf[:], pw[:], A.is_gt)
                nc.vector.tensor_tensor(cf[:], cf[:], ad[:], A.subtract)
                nc.vector.tensor_scalar(cf[:], cf[:], b0T[:, it:it + 1], None,
                                        A.subtract)
                nc.vector.tensor_copy(resI[:], cf[:])

            # window indices, all in fp32 (AP scalars must be fp32)
            gif = wpool.tile([128, NG], dt.float32, tag="gif")
            nc.vector.tensor_scalar(gif[:], iota7[:], G0fT[:, it:it + 1], None, A.add)
            x4f = wpool.tile([128, NG], dt.float32, tag="x4f")
            nc.vector.tensor_scalar(x4f[:], gif[:], 0.0625, None, A.mult)
            gci = wpool.tile([128, NG], dt.int32, tag="gci")
            nc.vector.tensor_copy(gci[:], x4f[:])
            gcf = wpool.tile([128, NG], dt.float32, tag="gcf")
            nc.vector.tensor_copy(gcf[:], gci[:])
            gad = wpool.tile([128, NG], dt.float32, tag="gad")
            nc.vector.tensor_tensor(gad[:], gcf[:], x4f[:], A.is_gt)
            nc.vector.tensor_tensor(x4f[:], gcf[:], gad[:], A.subtract)
            jf = wpool.tile([128, NG], dt.float32, tag="jf")
            nc.vector.scalar_tensor_tensor(jf[:], x4f[:], -16.0, gif[:], A.mult, A.add)
            j16 = wpool.tile([128, NG], dt.int16, tag="j16")
            nc.vector.tensor_copy(j16[:], jf[:])
            nc.vector.tensor_scalar(x4f[:], x4f[:], 86.0, q0mfT[:, it:it + 1],
                                    A.mult, A.add)
            e16 = wpool.tile([128, NG], dt.int16, tag="e16")
            nc.vector.tensor_copy(e16[:], x4f[:])

            g1 = gpool.tile([128, NG * 16 * 8], dt.float32, tag="g1")
            nc.gpsimd.ap_gather(g1[:], tab[:], e16[:], channels=128,
                                num_elems=2752, d=8, num_idxs=16 * NG)
            g2 = gpool.tile([128, NG * 16 * 8], dt.float32, tag="g2")
            nc.gpsimd.ap_gather(g2[:], mt2[:], j16[:], channels=128,
                                num_elems=16, d=8, num_idxs=16 * NG)

            # ---- stage B of previous tile (hides gather/PE latency) ----
            if prev is not None:
                stage_b(prev)

            nc.vector.tensor_tensor(g1[:], g1[:], g2[:], A.mult)
            pt = ppool.tile([128, NG * 8], dt.float32, tag="pt")
            mv4 = g1[:].rearrange("p (i k d) -> p k i d", i=NG, k=16, d=8)
            for k in range(16):
                nc.tensor.matmul(pt[:], st3[:, k, :], mv4[:, k, :, :],
                                 start=(k == 0), stop=(k == 15))
            prev = (it, pt, dc, sh)

        stage_b(prev)

        # ---- epilogue: penalty from max-key ----
        eg = cpool.tile([128, NT * 4], dt.float32)
        ev = eg[:].rearrange("p (c t) -> p c t", c=4)
        d2m, es0, er, val = ev[:, 0, :], ev[:, 1, :], ev[:, 2, :], ev[:, 3, :]
        nc.vector.tensor_scalar(d2m, res[:], -1.0, 16.0, A.mult, A.add)
        nc.scalar.activation(es0, d2m, mybir.ActivationFunctionType.Sqrt)
        nc.vector.reciprocal(er, es0)
        nc.vector.tensor_tensor(er, d2m, er, A.mult)
        nc.vector.tensor_tensor(er, er, es0, A.add)
        nc.vector.tensor_scalar(er, er, 0.5, None, A.mult)
        nc.vector.tensor_tensor(er, er, invpenT[:], A.mult)
        nc.vector.tensor_scalar(er, er, -1.0, 1.0, A.mult, A.add)
        nc.vector.tensor_scalar(val, res[:], 0.0, None, A.is_gt)
        out_t = cpool.tile([128, NT], dt.float32)
        nc.vector.tensor_tensor(out_t[:], er, val, A.mult)
        nc.sync.dma_start(out_dram, out_t[:])

    nc.compile()
    return nc


def kernel(traj, veh_att, raster, mapixes, dx, _trace=False):
    _install_ntff_hook()
    from concourse.bass_utils import run_bass_kernel_spmd

    traj = np.ascontiguousarray(traj, np.float32)
    veh_att = np.ascontiguousarray(veh_att, np.float32)
    raster = np.ascontiguousarray(raster, np.float32)
    mapixes = np.ascontiguousarray(mapixes).astype(np.int64)
    dxf = np.float32(np.asarray(dx).reshape(-1)[0])

    # ---- host layout prep ----
    # 24-row packed fp32 words: wm24[m, q, x], rows [24q, 24q+24)
    bits = (raster >= 0.5).astype(np.int32)
    bits = np.concatenate([bits, np.zeros((N_MAPS, 24 * 88 - MAP_H, MAP_W), np.int32)],
                          axis=1)
    wts = (1 << np.arange(24)).astype(np.int64)
    wm24 = (bits.reshape(N_MAPS, 88, 24, MAP_W).astype(np.int64)
            * wts[None, None, :, None]).sum(axis=2).astype(np.float32)  # [4,88,2048]

    # table: partition p owns 4-col groups G === p%16 (mod 16);
    # entry e = (x4l-2)*QB + q -> d = [c(4) x (w[q], w[q+1])]
    # x = 64*x4l + 4*(p%16) + c;  x4l in [2, 29] -> NE = 28*86 = 2408 (pad 2752)
    tabs = []
    x4l = np.arange(2, 30)          # 28 values
    qq = np.arange(QB)
    cc = np.arange(4)
    ss = np.arange(2)
    for m in range(N_MAPS):
        t = np.zeros((128, 2752, 8), np.float32)
        for p in range(128):
            x = 64 * x4l[:, None, None, None] + 4 * (p % 16) + cc[None, None, :, None]
            q = qq[None, :, None, None] + ss[None, None, None, :]
            v = wm24[m, q, x]                        # [28, 86, 4, 2]
            t[p, :28 * QB] = v.reshape(28 * QB, 8)
        tabs.append(t.reshape(128, 2752 * 8))

    # one-hot (by group-partition) mask table
    mt2 = np.zeros((128, 16, 8), np.float32)
    mt2[np.arange(128), np.arange(128) % 16, :] = 1
    mt2 = mt2.reshape(128, 128)

    # 16 one-hot stationaries: stat[p, 128k + 16*(p//16)+k] = 1
    stat = np.zeros((128, 16, 128), np.float32)
    pp = np.arange(128)
    for k in range(16):
        stat[pp, k, 16 * (pp // 16) + k] = 1
    stat = stat.reshape(128, 16 * 128)

    uu2, vv2 = np.meshgrid(_UU10, _VV20, indexing="ij")
    uu_rep = np.broadcast_to(uu2.reshape(1, P), (128, P)).astype(np.float32).copy()
    vv_rep = np.broadcast_to(vv2.reshape(1, P), (128, P)).astype(np.float32).copy()
    uusq = (uu_rep * uu_rep).astype(np.float32)
    vvsq = (vv_rep * vv_rep).astype(np.float32)
    iota7 = np.broadcast_to(np.arange(NG, dtype=np.float32)[None, :], (128, NG)).copy()
    iota28 = np.broadcast_to(np.arange(NC, dtype=np.int32)[None, :], (128, NC)).copy()
    dxrep = np.full((128, 1), dxf, np.float32)

    # ---- shard agents by map, 2 cores per map ----
    core_agents = [[] for _ in range(N_CORES)]
    for m in range(N_MAPS):
        ags = np.where(mapixes == m)[0]
        half = (len(ags) + 1) // 2
        core_agents[2 * m] = list(ags[:half])
        core_agents[2 * m + 1] = list(ags[half:])

    n_states = [len(a) * T for a in core_agents]
    n_tiles = max(1, int(np.ceil(max(n_states) / 128)))
    S = n_tiles * 128

    traj_flat = traj.reshape(NA * T, 4)
    in_maps = []
    state_maps = []
    for c in range(N_CORES):
        ags = core_agents[c]
        tr = np.zeros((S, 4), np.float32)
        at = np.zeros((S, 2), np.float32)
        smap = np.full(S, -1, np.int64)
        if ags:
            idx = np.array([(a * T + t) for a in ags for t in range(T)])
            tr[:len(idx)] = traj_flat[idx]
            at[:len(idx)] = veh_att[np.repeat(ags, T)]
            smap[:len(idx)] = idx
        pad = smap < 0
        tr[pad] = np.array([100.0, 100.0, 1.0, 0.0], np.float32)
        at[pad] = np.array([4.0, 2.0], np.float32)
        trt = tr.reshape(n_tiles, 128, 4).transpose(1, 0, 2)
        att2 = at.reshape(n_tiles, 128, 2).transpose(1, 0, 2)
        in_maps.append({
            "tab": tabs[c // 2], "mt2": mt2, "stat": stat,
            "uu": uu_rep, "vv": vv_rep, "uusq": uusq, "vvsq": vvsq,
            "iota7": iota7, "iota28": iota28, "dxrep": dxrep,
            "cxs": np.ascontiguousarray(trt[:, :, 0]),
            "cys": np.ascontiguousarray(trt[:, :, 1]),
            "hxs": np.ascontiguousarray(trt[:, :, 2]),
            "hys": np.ascontiguousarray(trt[:, :, 3]),
            "Ls": np.ascontiguousarray(att2[:, :, 0]),
            "Ws": np.ascontiguousarray(att2[:, :, 1]),
        })
        state_maps.append(smap)

    if n_tiles not in _PROGRAM_CACHE:
        _PROGRAM_CACHE[n_tiles] = _build_program(n_tiles)
    nc = _PROGRAM_CACHE[n_tiles]

    try:
        res = run_bass_kernel_spmd(nc, in_maps, list(range(N_CORES)), trace=_trace)
    except Exception:
        if not _trace:
            raise
        res = run_bass_kernel_spmd(nc, in_maps, list(range(N_CORES)), trace=False)
    kernel.last_results = res

    out = np.zeros(NA * T, np.float32)
    for c in range(N_CORES):
        o = res.results[c]["outsh"].T.reshape(-1)
        valid = state_maps[c] >= 0
        out[state_maps[c][valid]] = o[valid]
    return out


# revision 18
# speedup vs baseline: 5.8174x; 1.0804x over previous
"""Bass/TRN2 kernel for nn_EnvCollLoss (oriented-footprint raster collision loss).

v3: patch-gather design. Agents sharded by map (2 cores/map). Per state we
fetch a 28-col x 48-row window of the raster with SEVEN pooled ap_gather
indices (4-column groups x two 24-row fp32-packed words, d=8) instead of 200
per-point indices -- ap_gather cost is ~27ns/index, so this cuts the gather
from ~175us to ~11us per tile. A one-hot (by group-partition) gather + 16
accumulating one-hot-stationary matmuls reduce the 16 candidate partitions
into state-major PSUM. Each column's two 24-row words are fused into one
int32 covering rows [y0, y0+32); a 3-pass dense select over [128, 200x28]
(is_equal against an iota via stride-0 broadcast APs, mask-mult, or-reduce)
picks each point's column word, then a per-point shift extracts the bit.
Penalty uses dist = sqrt(min d2) over colliding points with d2 from constant
uu^2/vv^2 tables; per-state scalar math is hoisted into [128, n_tiles]-wide
prologue/epilogue. Stage-B work of tile N-1 is emitted between tile N's
gathers and matmuls so vector work hides gather/PE latency.
"""
import sys
import types
import numpy as np
from contextlib import ExitStack

NA, T = 256, 100
N_MAPS, MAP_H, MAP_W = 4, 2048, 2048
PU, PV = 10, 20
P = PU * PV  # 200
N_CORES = 8
NG = 7            # 4-col groups per state window (28 cols)
NC = 4 * NG       # 28 columns
QB = 86           # 24-row blocks per column (rows < 2064)

# jnp.linspace(-0.5, 0.5, 10/20, dtype=float32) exact values (validated vs jax)
_UU10 = np.array([-0.5, -0.3888889, -0.2777778, -0.16666667, -0.05555556,
                  0.05555556, 0.16666667, 0.2777778, 0.3888889, 0.5], dtype=np.float32)
_VV20 = np.linspace(-0.5, 0.5, 20, dtype=np.float32)


def _install_ntff_hook():
    import antenv
    if "antenv.axon_hooks" in sys.modules:
        return
    try:
        from trn_agent_boot.trn_boot import _ntff_profile_via_ctypes
        hook = _ntff_profile_via_ctypes("/opt/axon/libaxon_pjrt.so")
    except Exception:
        hook = None
    mod = types.ModuleType("antenv.axon_hooks")
    mod._hook = hook
    mod.get_axon_ntff_profile_hook = lambda: mod._hook
    mod.set_axon_ntff_profile_hook = lambda h: setattr(mod, "_hook", h)
    sys.modules["antenv.axon_hooks"] = mod
    antenv.axon_hooks = mod


_PROGRAM_CACHE = {}


def _build_program(n_tiles):
    import concourse.tile as tile
    from concourse import bacc, mybir
    from concourse.bass import broadcast_tensor_aps

    dt = mybir.dt
    A = mybir.AluOpType
    NT = n_tiles
    INV24 = float(np.float32(1.0) / np.float32(24.0))

    nc = bacc.Bacc("TRN2", target_bir_lowering=False, debug=False,
                   enable_asserts=False, num_devices=N_CORES)

    tab_in = nc.dram_tensor("tab", [128, 2752 * 8], dt.float32, kind="ExternalInput").ap()
    mt_in = nc.dram_tensor("mt2", [128, 16 * 8], dt.float32, kind="ExternalInput").ap()
    st_in = nc.dram_tensor("stat", [128, 16 * 128], dt.float32, kind="ExternalInput").ap()
    uu_in = nc.dram_tensor("uu", [128, P], dt.float32, kind="ExternalInput").ap()
    vv_in = nc.dram_tensor("vv", [128, P], dt.float32, kind="ExternalInput").ap()
    uq_in = nc.dram_tensor("uusq", [128, P], dt.float32, kind="ExternalInput").ap()
    vq_in = nc.dram_tensor("vvsq", [128, P], dt.float32, kind="ExternalInput").ap()
    i7_in = nc.dram_tensor("iota7", [128, NG], dt.float32, kind="ExternalInput").ap()
    i28_in = nc.dram_tensor("iota28", [128, NC], dt.int32, kind="ExternalInput").ap()
    dx_in = nc.dram_tensor("dxrep", [128, 1], dt.float32, kind="ExternalInput").ap()
    cx_in = nc.dram_tensor("cxs", [128, NT], dt.float32, kind="ExternalInput").ap()
    cy_in = nc.dram_tensor("cys", [128, NT], dt.float32, kind="ExternalInput").ap()
    hx_in = nc.dram_tensor("hxs", [128, NT], dt.float32, kind="ExternalInput").ap()
    hy_in = nc.dram_tensor("hys", [128, NT], dt.float32, kind="ExternalInput").ap()
    lL_in = nc.dram_tensor("Ls", [128, NT], dt.float32, kind="ExternalInput").ap()
    lW_in = nc.dram_tensor("Ws", [128, NT], dt.float32, kind="ExternalInput").ap()
    out_dram = nc.dram_tensor("outsh", [128, NT], dt.float32, kind="ExternalOutput").ap()

    with tile.TileContext(nc) as tc, ExitStack() as ctx:
        cpool = ctx.enter_context(tc.tile_pool(name="const", bufs=1))
        wpool = ctx.enter_context(tc.tile_pool(name="work", bufs=2))
        mpool = ctx.enter_context(tc.tile_pool(name="msel", bufs=1))
        gpool = ctx.enter_context(tc.tile_pool(name="gath", bufs=2))
        ppool = ctx.enter_context(tc.tile_pool(name="ps", bufs=2, space="PSUM"))

        def cload(name, shape, dtp, src):
            t = cpool.tile(shape, dtp, tag=name)
            nc.sync.dma_start(t[:], src)
            return t

        tab = cload("tab", [128, 2752 * 8], dt.float32, tab_in)
        mt2 = cload("mt2", [128, 16 * 8], dt.float32, mt_in)
        stat = cload("stat", [128, 16 * 128], dt.float32, st_in)
        uu = cload("uu", [128, P], dt.float32, uu_in)
        vv = cload("vv", [128, P], dt.float32, vv_in)
        uusq = cload("uusq", [128, P], dt.float32, uq_in)
        vvsq = cload("vvsq", [128, P], dt.float32, vq_in)
        iota7 = cload("iota7", [128, NG], dt.float32, i7_in)
        iota28 = cload("iota28", [128, NC], dt.int32, i28_in)
        dxrep = cload("dxrep", [128, 1], dt.float32, dx_in)
        cxT = cload("cxT", [128, NT], dt.float32, cx_in)
        cyT = cload("cyT", [128, NT], dt.float32, cy_in)
        hx0T = cload("hx0T", [128, NT], dt.float32, hx_in)
        hy0T = cload("hy0T", [128, NT], dt.float32, hy_in)
        LT = cload("LT", [128, NT], dt.float32, lL_in)
        WT = cload("WT", [128, NT], dt.float32, lW_in)

        invdx = cpool.tile([128, 1], dt.float32)
        nc.vector.reciprocal(invdx[:], dxrep[:])

        # ---- prologue: per-state scalars, [128, NT]-wide ----
        pg = cpool.tile([128, NT * 16], dt.float32)
        pv = pg[:].rearrange("p (c t) -> p c t", c=16)
        t0, t1, x2, delta = pv[:, 0, :], pv[:, 1, :], pv[:, 2, :], pv[:, 3, :]
        nc.vector.tensor_tensor(t0, hx0T[:], hx0T[:], A.mult)
        nc.vector.tensor_tensor(t1, hy0T[:], hy0T[:], A.mult)
        nc.vector.tensor_tensor(x2, t0, t1, A.add)
        nc.vector.tensor_scalar(delta, x2, -1.0, None, A.add)
        d5, dsq, den = pv[:, 4, :], pv[:, 5, :], pv[:, 6, :]
        nc.vector.tensor_scalar(d5, delta, 0.5, None, A.mult)
        nc.vector.tensor_tensor(dsq, delta, delta, A.mult)
        nc.vector.scalar_tensor_tensor(den, dsq, -0.125, d5, A.mult, A.add)
        nc.vector.tensor_scalar(den, den, 1.0, None, A.add)
        invn = cpool.tile([128, NT], dt.float32)
        nc.vector.reciprocal(invn[:], den)
        hxT = cpool.tile([128, NT], dt.float32)
        nc.vector.tensor_tensor(hxT[:], hx0T[:], invn[:], A.mult)
        hyT = cpool.tile([128, NT], dt.float32)
        nc.vector.tensor_tensor(hyT[:], hy0T[:], invn[:], A.mult)
        nhyT = cpool.tile([128, NT], dt.float32)
        nc.vector.tensor_scalar(nhyT[:], hyT[:], -1.0, None, A.mult)
        bxT = cpool.tile([128, NT], dt.float32)
        nc.vector.tensor_scalar(bxT[:], cxT[:], invdx[:], None, A.mult)
        byT = cpool.tile([128, NT], dt.float32)
        nc.vector.tensor_scalar(byT[:], cyT[:], invdx[:], None, A.mult)
        # -L^2, -W^2; pen = sqrt(L^2/4 + W^2/4); 1/pen
        tL, tW = pv[:, 7, :], pv[:, 8, :]
        nc.vector.tensor_tensor(tL, LT[:], LT[:], A.mult)
        nc.vector.tensor_tensor(tW, WT[:], WT[:], A.mult)
        nL2T = cpool.tile([128, NT], dt.float32)
        nc.vector.tensor_scalar(nL2T[:], tL, -1.0, None, A.mult)
        nW2T = cpool.tile([128, NT], dt.float32)
        nc.vector.tensor_scalar(nW2T[:], tW, -1.0, None, A.mult)
        p2, s0, rr = pv[:, 9, :], pv[:, 10, :], pv[:, 11, :]
        nc.vector.tensor_tensor(p2, tL, tW, A.add)
        nc.vector.tensor_scalar(p2, p2, 0.25, None, A.mult)
        nc.scalar.activation(s0, p2, mybir.ActivationFunctionType.Sqrt)
        nc.vector.reciprocal(rr, s0)
        nc.vector.tensor_tensor(rr, p2, rr, A.mult)
        nc.vector.tensor_tensor(rr, rr, s0, A.add)
        invpenT = cpool.tile([128, NT], dt.float32)
        nc.vector.tensor_scalar(invpenT[:], rr, 0.5, None, A.mult)
        nc.vector.reciprocal(invpenT[:], invpenT[:])

        # window bases: cxp=floor(cx/dx); x0=4*floor((cxp-12)/4); G0=(x0/4);
        # y0=floor(cy/dx)-12; q0=floor(y0/24); sh0=y0-24*q0; sh1=24-sh0
        fci = cpool.tile([128, NT], dt.int32, tag="fci")

        def _floor(dst, src):
            # exact floor via RNE-to-int then adjust (src >= 0 here)
            fr = pv[:, 15, :]
            nc.vector.tensor_copy(fci[:], src)
            nc.vector.tensor_copy(fr, fci[:])
            ad = pv[:, 9, :]
            nc.vector.tensor_tensor(ad, fr, src, A.is_gt)
            nc.vector.tensor_tensor(dst, fr, ad, A.subtract)

        cxpf, cypf = pv[:, 10, :], pv[:, 11, :]
        _floor(cxpf, bxT[:])
        _floor(cypf, byT[:])
        G0fT = cpool.tile([128, NT], dt.float32)
        nc.vector.tensor_scalar(G0fT[:], cxpf, -12.0, 0.25, A.add, A.mult)
        _floor(G0fT[:], G0fT[:])
        x0fT = cpool.tile([128, NT], dt.float32)
        nc.vector.tensor_scalar(x0fT[:], G0fT[:], 4.0, None, A.mult)
        y0fT = cpool.tile([128, NT], dt.float32)
        nc.vector.tensor_scalar(y0fT[:], cypf, -12.0, None, A.add)
        q0fT = cpool.tile([128, NT], dt.float32)
        nc.vector.tensor_scalar(q0fT[:], y0fT[:], INV24, None, A.mult)
        _floor(q0fT[:], q0fT[:])
        sh0fT = cpool.tile([128, NT], dt.float32)
        nc.vector.scalar_tensor_tensor(sh0fT[:], q0fT[:], -24.0, y0fT[:], A.mult, A.add)
        sh1fT = cpool.tile([128, NT], dt.float32)
        nc.vector.tensor_scalar(sh1fT[:], sh0fT[:], -1.0, 24.0, A.mult, A.add)
        q0mfT = cpool.tile([128, NT], dt.float32)        # q0 - 172 (e-base rebase)
        nc.vector.tensor_scalar(q0mfT[:], q0fT[:], -172.0, None, A.add)

        res = cpool.tile([128, NT], dt.float32)
        ones28 = cpool.tile([128, NC], dt.float32)
        nc.vector.memset(ones28[:], 1.0)
        st3 = stat[:].rearrange("p (k o) -> p k o", k=16)

        prev = None  # stage-B state of previous tile

        def stage_b1(pb):
            it, pt, dc, sh = pb
            # fuse each column's two 24-row words into one int32 window
            ptv = pt[:].rearrange("p (i c s) -> p (i c) s", i=NG, c=4, s=2)
            i0 = wpool.tile([128, NC], dt.int32, tag="i0")
            nc.vector.tensor_copy(i0[:], ptv[:, :, 0:1].rearrange("p q s -> p (q s)"))
            i1 = wpool.tile([128, NC], dt.int32, tag="i1")
            nc.vector.tensor_copy(i1[:], ptv[:, :, 1:2].rearrange("p q s -> p (q s)"))
            shbf = wpool.tile([128, NC], dt.float32, tag="shbf")
            shb = wpool.tile([128, NC], dt.int32, tag="shb")
            nc.vector.tensor_scalar(shbf[:], ones28[:], sh0fT[:, it:it + 1], None, A.mult)
            nc.vector.tensor_copy(shb[:], shbf[:])
            nc.vector.tensor_tensor(i0[:], i0[:], shb[:], A.logical_shift_right)
            nc.vector.tensor_scalar(shbf[:], ones28[:], sh1fT[:, it:it + 1], None, A.mult)
            nc.vector.tensor_copy(shb[:], shbf[:])
            nc.vector.tensor_tensor(i1[:], i1[:], shb[:], A.logical_shift_left)
            w32 = wpool.tile([128, NC], dt.int32, tag="w32")
            nc.vector.tensor_tensor(w32[:], i0[:], i1[:], A.bitwise_or)

            # dense column select, pass 1: M = (dc != iota28) on V
            M = mpool.tile([128, P * NC], dt.int32, tag="M")
            M3 = M[:].rearrange("p (t c) -> p t c", c=NC)
            dc3 = dc[:].rearrange("p (t o) -> p t o", o=1)
            io3 = iota28[:].rearrange("p (o c) -> p o c", o=1)
            dcB, ioB = broadcast_tensor_aps(dc3, io3)
            nc.vector.tensor_tensor(M3, dcB, ioB, A.not_equal)
            # pass 2 on the scalar engine: M -= 1 -> 0xFFFFFFFF where dc == c
            # (int32 `mult` routes through fp32 and mangles 32-bit words)
            nc.scalar.activation(M[:], M[:], mybir.ActivationFunctionType.Copy,
                                 bias=-1.0, scale=1.0)
            return (it, M, M3, w32, sh)

        def stage_b2(pb):
            it, M, M3, w32, sh = pb
            w3 = w32[:].rearrange("p (o c) -> p o c", o=1)
            w3B, M3B = broadcast_tensor_aps(w3, M3)
            nc.vector.tensor_tensor(M3, M3B, w3B, A.bitwise_and)
            R = wpool.tile([128, P], dt.int32, tag="R")
            nc.vector.tensor_reduce(R[:], M3, mybir.AxisListType.X, A.bitwise_or)

            # bit extract + key + reduce
            nc.vector.tensor_tensor(R[:], R[:], sh[:], A.logical_shift_right)
            nc.vector.tensor_scalar(R[:], R[:], 1, 1, A.bitwise_and, A.bitwise_xor)
            cbf = wpool.tile([128, P], dt.float32, tag="cbf")
            nc.vector.tensor_copy(cbf[:], R[:])
            tw = wpool.tile([128, P], dt.float32, tag="tw")
            nc.vector.tensor_scalar(tw[:], vvsq[:], nW2T[:, it:it + 1], 16.0,
                                    A.mult, A.add)
            ckey = wpool.tile([128, P], dt.float32, tag="ckey")
            nc.vector.scalar_tensor_tensor(ckey[:], uusq[:], nL2T[:, it:it + 1],
                                           tw[:], A.mult, A.add)
            nc.vector.tensor_tensor(ckey[:], ckey[:], cbf[:], A.mult)
            mx8 = wpool.tile([128, 8], dt.float32, tag="mx8")
            nc.vector.max(mx8[:], ckey[:])
            nc.vector.tensor_copy(res[:, it:it + 1], mx8[:, 0:1])

        for it in range(n_tiles):
            Lc, Wc = LT[:, it:it + 1], WT[:, it:it + 1]
            hxc, hyc, nhyc = hxT[:, it:it + 1], hyT[:, it:it + 1], nhyT[:, it:it + 1]

            # ---- stage A: per-point pixel math + window indices ----
            bu = wpool.tile([128, P], dt.float32, tag="bu")
            nc.scalar.activation(bu[:], uu[:], mybir.ActivationFunctionType.Copy,
                                 bias=0.0, scale=Lc)
            bv = wpool.tile([128, P], dt.float32, tag="bv")
            nc.scalar.activation(bv[:], vv[:], mybir.ActivationFunctionType.Copy,
                                 bias=0.0, scale=Wc)
            t1t = wpool.tile([128, P], dt.float32, tag="t1t")
            nc.scalar.activation(t1t[:], bu[:], mybir.ActivationFunctionType.Copy,
                                 bias=0.0, scale=hxc)
            t2t = wpool.tile([128, P], dt.float32, tag="t2t")
            nc.scalar.activation(t2t[:], bu[:], mybir.ActivationFunctionType.Copy,
                                 bias=0.0, scale=hyc)
            ox = wpool.tile([128, P], dt.float32, tag="ox")
            nc.vector.scalar_tensor_tensor(ox[:], bv[:], nhyc, t1t[:], A.mult, A.add)
            oy = wpool.tile([128, P], dt.float32, tag="oy")
            nc.vector.scalar_tensor_tensor(oy[:], bv[:], hxc, t2t[:], A.mult, A.add)

            # previous tile's w32 build + mask pass 1 (V) + mask pass 2 (S)
            pb1 = stage_b1(prev) if prev is not None else None

            dc = wpool.tile([128, P], dt.int32, tag="dc")
            sh = wpool.tile([128, P], dt.int32, tag="sh")
            for (ov, bc, b0T, resI) in ((ox, bxT, x0fT, dc), (oy, byT, y0fT, sh)):
                pw = wpool.tile([128, P], dt.float32, tag="pw")
                nc.vector.tensor_scalar(pw[:], ov[:], invdx[:], bc[:, it:it + 1],
                                        A.mult, A.add)
                ci = wpool.tile([128, P], dt.int32, tag="ci")
                nc.vector.tensor_copy(ci[:], pw[:])
                cf = wpool.tile([128, P], dt.float32, tag="cf")
                nc.vector.tensor_copy(cf[:], ci[:])
                ad = wpool.tile([128, P], dt.float32, tag="ad")
                nc.vector.tensor_tensor(ad[:], cf[:], pw[:], A.is_gt)
                nc.vector.tensor_tensor(cf[:], cf[:], ad[:], A.subtract)
                nc.vector.tensor_scalar(cf[:], cf[:], b0T[:, it:it + 1], None,
                                        A.subtract)
                nc.vector.tensor_copy(resI[:], cf[:])

            # window indices, all in fp32 (AP scalars must be fp32)
            gif = wpool.tile([128, NG], dt.float32, tag="gif")
            nc.vector.tensor_scalar(gif[:], iota7[:], G0fT[:, it:it + 1], None, A.add)
            x4f = wpool.tile([128, NG], dt.float32, tag="x4f")
            nc.vector.tensor_scalar(x4f[:], gif[:], 0.0625, None, A.mult)
            gci = wpool.tile([128, NG], dt.int32, tag="gci")
            nc.vector.tensor_copy(gci[:], x4f[:])
            gcf = wpool.tile([128, NG], dt.float32, tag="gcf")
            nc.vector.tensor_copy(gcf[:], gci[:])
            gad = wpool.tile([128, NG], dt.float32, tag="gad")
            nc.vector.tensor_tensor(gad[:], gcf[:], x4f[:], A.is_gt)
            nc.vector.tensor_tensor(x4f[:], gcf[:], gad[:], A.subtract)
            jf = wpool.tile([128, NG], dt.float32, tag="jf")
            nc.vector.scalar_tensor_tensor(jf[:], x4f[:], -16.0, gif[:], A.mult, A.add)
            j16 = wpool.tile([128, NG], dt.int16, tag="j16")
            nc.vector.tensor_copy(j16[:], jf[:])
            nc.vector.tensor_scalar(x4f[:], x4f[:], 86.0, q0mfT[:, it:it + 1],
                                    A.mult, A.add)
            e16 = wpool.tile([128, NG], dt.int16, tag="e16")
            nc.vector.tensor_copy(e16[:], x4f[:])

            g1 = gpool.tile([128, NG * 16 * 8], dt.float32, tag="g1")
            nc.gpsimd.ap_gather(g1[:], tab[:], e16[:], channels=128,
                                num_elems=2752, d=8, num_idxs=16 * NG)
            g2 = gpool.tile([128, NG * 16 * 8], dt.float32, tag="g2")
            nc.gpsimd.ap_gather(g2[:], mt2[:], j16[:], channels=128,
                                num_elems=16, d=8, num_idxs=16 * NG)

            # previous tile's select/extract/key (waits on the scalar mask pass)
            if pb1 is not None:
                stage_b2(pb1)

            nc.vector.tensor_tensor(g1[:], g1[:], g2[:], A.mult)
            pt = ppool.tile([128, NG * 8], dt.float32, tag="pt")
            mv4 = g1[:].rearrange("p (i k d) -> p k i d", i=NG, k=16, d=8)
            for k in range(16):
                nc.tensor.matmul(pt[:], st3[:, k, :], mv4[:, k, :, :],
                                 start=(k == 0), stop=(k == 15))
            prev = (it, pt, dc, sh)

        stage_b2(stage_b1(prev))

        # ---- epilogue: penalty from max-key ----
        eg = cpool.tile([128, NT * 4], dt.float32)
        ev = eg[:].rearrange("p (c t) -> p c t", c=4)
        d2m, es0, er, val = ev[:, 0, :], ev[:, 1, :], ev[:, 2, :], ev[:, 3, :]
        nc.vector.tensor_scalar(d2m, res[:], -1.0, 16.0, A.mult, A.add)
        nc.scalar.activation(es0, d2m, mybir.ActivationFunctionType.Sqrt)
        nc.vector.reciprocal(er, es0)
        nc.vector.tensor_tensor(er, d2m, er, A.mult)
        nc.vector.tensor_tensor(er, er, es0, A.add)
        nc.vector.tensor_scalar(er, er, 0.5, None, A.mult)
        nc.vector.tensor_tensor(er, er, invpenT[:], A.mult)
        nc.vector.tensor_scalar(er, er, -1.0, 1.0, A.mult, A.add)
        nc.vector.tensor_scalar(val, res[:], 0.0, None, A.is_gt)
        out_t = cpool.tile([128, NT], dt.float32)
        nc.vector.tensor_tensor(out_t[:], er, val, A.mult)
        nc.sync.dma_start(out_dram, out_t[:])

    nc.compile()
    return nc


def kernel(traj, veh_att, raster, mapixes, dx, _trace=False):
    _install_ntff_hook()
    from concourse.bass_utils import run_bass_kernel_spmd

    traj = np.ascontiguousarray(traj, np.float32)
    veh_att = np.ascontiguousarray(veh_att, np.float32)
    raster = np.ascontiguousarray(raster, np.float32)
    mapixes = np.ascontiguousarray(mapixes).astype(np.int64)
    dxf = np.float32(np.asarray(dx).reshape(-1)[0])

    # ---- host layout prep ----
    # 24-row packed fp32 words: wm24[m, q, x], rows [24q, 24q+24)
    bits = (raster >= 0.5).astype(np.int32)
    bits = np.concatenate([bits, np.zeros((N_MAPS, 24 * 88 - MAP_H, MAP_W), np.int32)],
                          axis=1)
    wts = (1 << np.arange(24)).astype(np.int64)
    wm24 = (bits.reshape(N_MAPS, 88, 24, MAP_W).astype(np.int64)
            * wts[None, None, :, None]).sum(axis=2).astype(np.float32)  # [4,88,2048]

    # table: partition p owns 4-col groups G === p%16 (mod 16);
    # entry e = (x4l-2)*QB + q -> d = [c(4) x (w[q], w[q+1])]
    # x = 64*x4l + 4*(p%16) + c;  x4l in [2, 29] -> NE = 28*86 = 2408 (pad 2752)
    tabs = []
    x4l = np.arange(2, 30)          # 28 values
    qq = np.arange(QB)
    cc = np.arange(4)
    ss = np.arange(2)
    for m in range(N_MAPS):
        t = np.zeros((128, 2752, 8), np.float32)
        for p in range(128):
            x = 64 * x4l[:, None, None, None] + 4 * (p % 16) + cc[None, None, :, None]
            q = qq[None, :, None, None] + ss[None, None, None, :]
            v = wm24[m, q, x]                        # [28, 86, 4, 2]
            t[p, :28 * QB] = v.reshape(28 * QB, 8)
        tabs.append(t.reshape(128, 2752 * 8))

    # one-hot (by group-partition) mask table
    mt2 = np.zeros((128, 16, 8), np.float32)
    mt2[np.arange(128), np.arange(128) % 16, :] = 1
    mt2 = mt2.reshape(128, 128)

    # 16 one-hot stationaries: stat[p, 128k + 16*(p//16)+k] = 1
    stat = np.zeros((128, 16, 128), np.float32)
    pp = np.arange(128)
    for k in range(16):
        stat[pp, k, 16 * (pp // 16) + k] = 1
    stat = stat.reshape(128, 16 * 128)

    uu2, vv2 = np.meshgrid(_UU10, _VV20, indexing="ij")
    uu_rep = np.broadcast_to(uu2.reshape(1, P), (128, P)).astype(np.float32).copy()
    vv_rep = np.broadcast_to(vv2.reshape(1, P), (128, P)).astype(np.float32).copy()
    uusq = (uu_rep * uu_rep).astype(np.float32)
    vvsq = (vv_rep * vv_rep).astype(np.float32)
    iota7 = np.broadcast_to(np.arange(NG, dtype=np.float32)[None, :], (128, NG)).copy()
    iota28 = np.broadcast_to(np.arange(NC, dtype=np.int32)[None, :], (128, NC)).copy()
    dxrep = np.full((128, 1), dxf, np.float32)

    # ---- shard agents by map, 2 cores per map ----
    core_agents = [[] for _ in range(N_CORES)]
    for m in range(N_MAPS):
        ags = np.where(mapixes == m)[0]
        half = (len(ags) + 1) // 2
        core_agents[2 * m] = list(ags[:half])
        core_agents[2 * m + 1] = list(ags[half:])

    n_states = [len(a) * T for a in core_agents]
    n_tiles = max(1, int(np.ceil(max(n_states) / 128)))
    S = n_tiles * 128

    traj_flat = traj.reshape(NA * T, 4)
    in_maps = []
    state_maps = []
    for c in range(N_CORES):
        ags = core_agents[c]
        tr = np.zeros((S, 4), np.float32)
        at = np.zeros((S, 2), np.float32)
        smap = np.full(S, -1, np.int64)
        if ags:
            idx = np.array([(a * T + t) for a in ags for t in range(T)])
            tr[:len(idx)] = traj_flat[idx]
            at[:len(idx)] = veh_att[np.repeat(ags, T)]
            smap[:len(idx)] = idx
        pad = smap < 0
        tr[pad] = np.array([100.0, 100.0, 1.0, 0.0], np.float32)
        at[pad] = np.array([4.0, 2.0], np.float32)
        trt = tr.reshape(n_tiles, 128, 4).transpose(1, 0, 2)
        att2 = at.reshape(n_tiles, 128, 2).transpose(1, 0, 2)
        in_maps.append({
            "tab": tabs[c // 2], "mt2": mt2, "stat": stat,
            "uu": uu_rep, "vv": vv_rep, "uusq": uusq, "vvsq": vvsq,
            "iota7": iota7, "iota28": iota28, "dxrep": dxrep,
            "cxs": np.ascontiguousarray(trt[:, :, 0]),
            "cys": np.ascontiguousarray(trt[:, :, 1]),
            "hxs": np.ascontiguousarray(trt[:, :, 2]),
            "hys": np.ascontiguousarray(trt[:, :, 3]),
            "Ls": np.ascontiguousarray(att2[:, :, 0]),
            "Ws": np.ascontiguousarray(att2[:, :, 1]),
        })
        state_maps.append(smap)

    if n_tiles not in _PROGRAM_CACHE:
        _PROGRAM_CACHE[n_tiles] = _build_program(n_tiles)
    nc = _PROGRAM_CACHE[n_tiles]

    try:
        res = run_bass_kernel_spmd(nc, in_maps, list(range(N_CORES)), trace=_trace)
    except Exception:
        if not _trace:
            raise
        res = run_bass_kernel_spmd(nc, in_maps, list(range(N_CORES)), trace=False)
    kernel.last_results = res

    out = np.zeros(NA * T, np.float32)
    for c in range(N_CORES):
        o = res.results[c]["outsh"].T.reshape(-1)
        valid = state_maps[c] >= 0
        out[state_maps[c][valid]] = o[valid]
    return out


# revision 19
# speedup vs baseline: 6.1298x; 1.0537x over previous
"""Bass/TRN2 kernel for nn_EnvCollLoss (oriented-footprint raster collision loss).

v3: patch-gather design. Agents sharded by map (2 cores/map). Per state we
fetch a 28-col x 48-row window of the raster with SEVEN pooled ap_gather
indices (4-column groups x two 24-row fp32-packed words, d=8) instead of 200
per-point indices -- ap_gather cost is ~27ns/index, so this cuts the gather
from ~175us to ~11us per tile. A one-hot (by group-partition) gather + 16
accumulating one-hot-stationary matmuls reduce the 16 candidate partitions
into state-major PSUM. Each column's two 24-row words are fused into one
int32 covering rows [y0, y0+32); a 3-pass dense select over [128, 200x28]
(is_equal against an iota via stride-0 broadcast APs, mask-mult, or-reduce)
picks each point's column word, then a per-point shift extracts the bit.
Penalty uses dist = sqrt(min d2) over colliding points with d2 from constant
uu^2/vv^2 tables; per-state scalar math is hoisted into [128, n_tiles]-wide
prologue/epilogue. Stage-B work of tile N-1 is emitted between tile N's
gathers and matmuls so vector work hides gather/PE latency.
"""
import sys
import types
import numpy as np
from contextlib import ExitStack

NA, T = 256, 100
N_MAPS, MAP_H, MAP_W = 4, 2048, 2048
PU, PV = 10, 20
P = PU * PV  # 200
N_CORES = 8
NG = 7            # 4-col groups per state window (28 cols)
NC = 4 * NG       # 28 columns
QB = 86           # 24-row blocks per column (rows < 2064)

# jnp.linspace(-0.5, 0.5, 10/20, dtype=float32) exact values (validated vs jax)
_UU10 = np.array([-0.5, -0.3888889, -0.2777778, -0.16666667, -0.05555556,
                  0.05555556, 0.16666667, 0.2777778, 0.3888889, 0.5], dtype=np.float32)
_VV20 = np.linspace(-0.5, 0.5, 20, dtype=np.float32)


def _install_ntff_hook():
    import antenv
    if "antenv.axon_hooks" in sys.modules:
        return
    try:
        from trn_agent_boot.trn_boot import _ntff_profile_via_ctypes
        hook = _ntff_profile_via_ctypes("/opt/axon/libaxon_pjrt.so")
    except Exception:
        hook = None
    mod = types.ModuleType("antenv.axon_hooks")
    mod._hook = hook
    mod.get_axon_ntff_profile_hook = lambda: mod._hook
    mod.set_axon_ntff_profile_hook = lambda h: setattr(mod, "_hook", h)
    sys.modules["antenv.axon_hooks"] = mod
    antenv.axon_hooks = mod


_PROGRAM_CACHE = {}


def _build_program(n_tiles):
    import concourse.tile as tile
    from concourse import bacc, mybir
    from concourse.bass import broadcast_tensor_aps

    dt = mybir.dt
    A = mybir.AluOpType
    NT = n_tiles
    INV24 = float(np.float32(1.0) / np.float32(24.0))

    nc = bacc.Bacc("TRN2", target_bir_lowering=False, debug=False,
                   enable_asserts=False, num_devices=N_CORES)

    tab_in = nc.dram_tensor("tab", [128, 2752 * 8], dt.float32, kind="ExternalInput").ap()
    mt_in = nc.dram_tensor("mt2", [128, 16 * 8], dt.float32, kind="ExternalInput").ap()
    st_in = nc.dram_tensor("stat", [128, 16 * 128], dt.float32, kind="ExternalInput").ap()
    uu_in = nc.dram_tensor("uu", [128, P], dt.float32, kind="ExternalInput").ap()
    vv_in = nc.dram_tensor("vv", [128, P], dt.float32, kind="ExternalInput").ap()
    uq_in = nc.dram_tensor("uusq", [128, P], dt.float32, kind="ExternalInput").ap()
    vq_in = nc.dram_tensor("vvsq", [128, P], dt.float32, kind="ExternalInput").ap()
    i7_in = nc.dram_tensor("iota7", [128, NG], dt.float32, kind="ExternalInput").ap()
    i28_in = nc.dram_tensor("iota28", [128, NC], dt.int16, kind="ExternalInput").ap()
    dx_in = nc.dram_tensor("dxrep", [128, 1], dt.float32, kind="ExternalInput").ap()
    cx_in = nc.dram_tensor("cxs", [128, NT], dt.float32, kind="ExternalInput").ap()
    cy_in = nc.dram_tensor("cys", [128, NT], dt.float32, kind="ExternalInput").ap()
    hx_in = nc.dram_tensor("hxs", [128, NT], dt.float32, kind="ExternalInput").ap()
    hy_in = nc.dram_tensor("hys", [128, NT], dt.float32, kind="ExternalInput").ap()
    lL_in = nc.dram_tensor("Ls", [128, NT], dt.float32, kind="ExternalInput").ap()
    lW_in = nc.dram_tensor("Ws", [128, NT], dt.float32, kind="ExternalInput").ap()
    out_dram = nc.dram_tensor("outsh", [128, NT], dt.float32, kind="ExternalOutput").ap()

    with tile.TileContext(nc) as tc, ExitStack() as ctx:
        cpool = ctx.enter_context(tc.tile_pool(name="const", bufs=1))
        wpool = ctx.enter_context(tc.tile_pool(name="work", bufs=2))
        mpool = ctx.enter_context(tc.tile_pool(name="msel", bufs=1))
        gpool = ctx.enter_context(tc.tile_pool(name="gath", bufs=2))
        ppool = ctx.enter_context(tc.tile_pool(name="ps", bufs=2, space="PSUM"))

        def cload(name, shape, dtp, src):
            t = cpool.tile(shape, dtp, tag=name)
            nc.sync.dma_start(t[:], src)
            return t

        tab = cload("tab", [128, 2752 * 8], dt.float32, tab_in)
        mt2 = cload("mt2", [128, 16 * 8], dt.float32, mt_in)
        stat = cload("stat", [128, 16 * 128], dt.float32, st_in)
        uu = cload("uu", [128, P], dt.float32, uu_in)
        vv = cload("vv", [128, P], dt.float32, vv_in)
        uusq = cload("uusq", [128, P], dt.float32, uq_in)
        vvsq = cload("vvsq", [128, P], dt.float32, vq_in)
        iota7 = cload("iota7", [128, NG], dt.float32, i7_in)
        iota28 = cload("iota28", [128, NC], dt.int16, i28_in)
        dxrep = cload("dxrep", [128, 1], dt.float32, dx_in)
        cxT = cload("cxT", [128, NT], dt.float32, cx_in)
        cyT = cload("cyT", [128, NT], dt.float32, cy_in)
        hx0T = cload("hx0T", [128, NT], dt.float32, hx_in)
        hy0T = cload("hy0T", [128, NT], dt.float32, hy_in)
        LT = cload("LT", [128, NT], dt.float32, lL_in)
        WT = cload("WT", [128, NT], dt.float32, lW_in)

        invdx = cpool.tile([128, 1], dt.float32)
        nc.vector.reciprocal(invdx[:], dxrep[:])

        # ---- prologue: per-state scalars, [128, NT]-wide ----
        pg = cpool.tile([128, NT * 16], dt.float32)
        pv = pg[:].rearrange("p (c t) -> p c t", c=16)
        t0, t1, x2, delta = pv[:, 0, :], pv[:, 1, :], pv[:, 2, :], pv[:, 3, :]
        nc.vector.tensor_tensor(t0, hx0T[:], hx0T[:], A.mult)
        nc.vector.tensor_tensor(t1, hy0T[:], hy0T[:], A.mult)
        nc.vector.tensor_tensor(x2, t0, t1, A.add)
        nc.vector.tensor_scalar(delta, x2, -1.0, None, A.add)
        d5, dsq, den = pv[:, 4, :], pv[:, 5, :], pv[:, 6, :]
        nc.vector.tensor_scalar(d5, delta, 0.5, None, A.mult)
        nc.vector.tensor_tensor(dsq, delta, delta, A.mult)
        nc.vector.scalar_tensor_tensor(den, dsq, -0.125, d5, A.mult, A.add)
        nc.vector.tensor_scalar(den, den, 1.0, None, A.add)
        invn = cpool.tile([128, NT], dt.float32)
        nc.vector.reciprocal(invn[:], den)
        hxT = cpool.tile([128, NT], dt.float32)
        nc.vector.tensor_tensor(hxT[:], hx0T[:], invn[:], A.mult)
        hyT = cpool.tile([128, NT], dt.float32)
        nc.vector.tensor_tensor(hyT[:], hy0T[:], invn[:], A.mult)
        nhyT = cpool.tile([128, NT], dt.float32)
        nc.vector.tensor_scalar(nhyT[:], hyT[:], -1.0, None, A.mult)
        bxT = cpool.tile([128, NT], dt.float32)
        nc.vector.tensor_scalar(bxT[:], cxT[:], invdx[:], None, A.mult)
        byT = cpool.tile([128, NT], dt.float32)
        nc.vector.tensor_scalar(byT[:], cyT[:], invdx[:], None, A.mult)
        # -L^2, -W^2; pen = sqrt(L^2/4 + W^2/4); 1/pen
        tL, tW = pv[:, 7, :], pv[:, 8, :]
        nc.vector.tensor_tensor(tL, LT[:], LT[:], A.mult)
        nc.vector.tensor_tensor(tW, WT[:], WT[:], A.mult)
        nL2T = cpool.tile([128, NT], dt.float32)
        nc.vector.tensor_scalar(nL2T[:], tL, -1.0, None, A.mult)
        nW2T = cpool.tile([128, NT], dt.float32)
        nc.vector.tensor_scalar(nW2T[:], tW, -1.0, None, A.mult)
        p2, s0, rr = pv[:, 9, :], pv[:, 10, :], pv[:, 11, :]
        nc.vector.tensor_tensor(p2, tL, tW, A.add)
        nc.vector.tensor_scalar(p2, p2, 0.25, None, A.mult)
        nc.scalar.activation(s0, p2, mybir.ActivationFunctionType.Sqrt)
        nc.vector.reciprocal(rr, s0)
        nc.vector.tensor_tensor(rr, p2, rr, A.mult)
        nc.vector.tensor_tensor(rr, rr, s0, A.add)
        invpenT = cpool.tile([128, NT], dt.float32)
        nc.vector.tensor_scalar(invpenT[:], rr, 0.5, None, A.mult)
        nc.vector.reciprocal(invpenT[:], invpenT[:])

        # window bases: cxp=floor(cx/dx); x0=4*floor((cxp-12)/4); G0=(x0/4);
        # y0=floor(cy/dx)-12; q0=floor(y0/24); sh0=y0-24*q0; sh1=24-sh0
        fci = cpool.tile([128, NT], dt.int32, tag="fci")

        def _floor(dst, src):
            # exact floor via RNE-to-int then adjust (src >= 0 here)
            fr = pv[:, 15, :]
            nc.vector.tensor_copy(fci[:], src)
            nc.vector.tensor_copy(fr, fci[:])
            ad = pv[:, 9, :]
            nc.vector.tensor_tensor(ad, fr, src, A.is_gt)
            nc.vector.tensor_tensor(dst, fr, ad, A.subtract)

        cxpf, cypf = pv[:, 10, :], pv[:, 11, :]
        _floor(cxpf, bxT[:])
        _floor(cypf, byT[:])
        G0fT = cpool.tile([128, NT], dt.float32)
        nc.vector.tensor_scalar(G0fT[:], cxpf, -12.0, 0.25, A.add, A.mult)
        _floor(G0fT[:], G0fT[:])
        x0fT = cpool.tile([128, NT], dt.float32)
        nc.vector.tensor_scalar(x0fT[:], G0fT[:], 4.0, None, A.mult)
        y0fT = cpool.tile([128, NT], dt.float32)
        nc.vector.tensor_scalar(y0fT[:], cypf, -12.0, None, A.add)
        q0fT = cpool.tile([128, NT], dt.float32)
        nc.vector.tensor_scalar(q0fT[:], y0fT[:], INV24, None, A.mult)
        _floor(q0fT[:], q0fT[:])
        sh0fT = cpool.tile([128, NT], dt.float32)
        nc.vector.scalar_tensor_tensor(sh0fT[:], q0fT[:], -24.0, y0fT[:], A.mult, A.add)
        sh1fT = cpool.tile([128, NT], dt.float32)
        nc.vector.tensor_scalar(sh1fT[:], sh0fT[:], -1.0, 24.0, A.mult, A.add)
        q0mfT = cpool.tile([128, NT], dt.float32)        # q0 - 172 (e-base rebase)
        nc.vector.tensor_scalar(q0mfT[:], q0fT[:], -172.0, None, A.add)
        # per-tile shift amounts broadcast along the 28 columns, hoisted
        ones28 = cpool.tile([128, NC], dt.float32)
        nc.vector.memset(ones28[:], 1.0)
        shbf_all = cpool.tile([128, NT * NC], dt.float32)
        sb3 = shbf_all[:].rearrange("p (t c) -> p t c", c=NC)
        sh03 = sh0fT[:].rearrange("p (t o) -> p t o", o=1)
        on3 = ones28[:].rearrange("p (o c) -> p o c", o=1)
        sh0B, onB = broadcast_tensor_aps(sh03, on3)
        nc.vector.tensor_tensor(sb3, sh0B, onB, A.mult)
        shb0_all = cpool.tile([128, NT * NC], dt.int32)
        nc.vector.tensor_copy(shb0_all[:], shbf_all[:])
        sh13 = sh1fT[:].rearrange("p (t o) -> p t o", o=1)
        sh1B, onB2 = broadcast_tensor_aps(sh13, on3)
        nc.vector.tensor_tensor(sb3, sh1B, onB2, A.mult)
        shb1_all = cpool.tile([128, NT * NC], dt.int32)
        nc.vector.tensor_copy(shb1_all[:], shbf_all[:])

        res = cpool.tile([128, NT], dt.float32)
        st3 = stat[:].rearrange("p (k o) -> p k o", k=16)

        prev = None  # stage-B state of previous tile

        def stage_b1(pb):
            it, pt, dc, sh = pb
            # fuse each column's two 24-row words into one int32 window
            ptv = pt[:].rearrange("p (i c s) -> p (i c) s", i=NG, c=4, s=2)
            i0 = wpool.tile([128, NC], dt.int32, tag="i0")
            nc.vector.tensor_copy(i0[:], ptv[:, :, 0:1].rearrange("p q s -> p (q s)"))
            i1 = wpool.tile([128, NC], dt.int32, tag="i1")
            nc.vector.tensor_copy(i1[:], ptv[:, :, 1:2].rearrange("p q s -> p (q s)"))
            nc.vector.tensor_tensor(i0[:], i0[:],
                                    shb0_all[:, it * NC:(it + 1) * NC],
                                    A.logical_shift_right)
            nc.vector.tensor_tensor(i1[:], i1[:],
                                    shb1_all[:, it * NC:(it + 1) * NC],
                                    A.logical_shift_left)
            w32 = wpool.tile([128, NC], dt.int32, tag="w32")
            nc.vector.tensor_tensor(w32[:], i0[:], i1[:], A.bitwise_or)

            # dense column select, pass 1: M16 = (dc != iota28) on V (int16, 2x)
            M16 = mpool.tile([128, P * NC], dt.int16, tag="M16")
            M163 = M16[:].rearrange("p (t c) -> p t c", c=NC)
            dc3 = dc[:].rearrange("p (t o) -> p t o", o=1)
            io3 = iota28[:].rearrange("p (o c) -> p o c", o=1)
            dcB, ioB = broadcast_tensor_aps(dc3, io3)
            nc.vector.tensor_tensor(M163, dcB, ioB, A.not_equal)
            # pass 2 on the scalar engine: M32 = M16 - 1 (0xFFFFFFFF where dc == c)
            # (int32 `mult` routes through fp32 and mangles 32-bit words)
            M = mpool.tile([128, P * NC], dt.int32, tag="M")
            M3 = M[:].rearrange("p (t c) -> p t c", c=NC)
            nc.scalar.activation(M[:], M16[:], mybir.ActivationFunctionType.Copy,
                                 bias=-1.0, scale=1.0)
            return (it, M, M3, w32, sh)

        def stage_b2(pb):
            it, M, M3, w32, sh = pb
            w3 = w32[:].rearrange("p (o c) -> p o c", o=1)
            w3B, M3B = broadcast_tensor_aps(w3, M3)
            nc.vector.tensor_tensor(M3, M3B, w3B, A.bitwise_and)
            R = wpool.tile([128, P], dt.int32, tag="R")
            nc.vector.tensor_reduce(R[:], M3, mybir.AxisListType.X, A.bitwise_or)

            # bit extract + key + reduce
            nc.vector.tensor_tensor(R[:], R[:], sh[:], A.logical_shift_right)
            nc.vector.tensor_scalar(R[:], R[:], 1, 1, A.bitwise_and, A.bitwise_xor)
            cbf = wpool.tile([128, P], dt.float32, tag="cbf")
            nc.vector.tensor_copy(cbf[:], R[:])
            tw = wpool.tile([128, P], dt.float32, tag="tw")
            nc.vector.tensor_scalar(tw[:], vvsq[:], nW2T[:, it:it + 1], 16.0,
                                    A.mult, A.add)
            ckey = wpool.tile([128, P], dt.float32, tag="ckey")
            nc.vector.scalar_tensor_tensor(ckey[:], uusq[:], nL2T[:, it:it + 1],
                                           tw[:], A.mult, A.add)
            nc.vector.tensor_tensor(ckey[:], ckey[:], cbf[:], A.mult)
            mx8 = wpool.tile([128, 8], dt.float32, tag="mx8")
            nc.vector.max(mx8[:], ckey[:])
            nc.vector.tensor_copy(res[:, it:it + 1], mx8[:, 0:1])

        for it in range(n_tiles):
            Lc, Wc = LT[:, it:it + 1], WT[:, it:it + 1]
            hxc, hyc, nhyc = hxT[:, it:it + 1], hyT[:, it:it + 1], nhyT[:, it:it + 1]

            # ---- stage A: per-point pixel math + window indices ----
            bu = wpool.tile([128, P], dt.float32, tag="bu")
            nc.scalar.activation(bu[:], uu[:], mybir.ActivationFunctionType.Copy,
                                 bias=0.0, scale=Lc)
            bv = wpool.tile([128, P], dt.float32, tag="bv")
            nc.scalar.activation(bv[:], vv[:], mybir.ActivationFunctionType.Copy,
                                 bias=0.0, scale=Wc)
            t1t = wpool.tile([128, P], dt.float32, tag="t1t")
            nc.scalar.activation(t1t[:], bu[:], mybir.ActivationFunctionType.Copy,
                                 bias=0.0, scale=hxc)
            t2t = wpool.tile([128, P], dt.float32, tag="t2t")
            nc.scalar.activation(t2t[:], bu[:], mybir.ActivationFunctionType.Copy,
                                 bias=0.0, scale=hyc)
            # previous tile's w32 build + mask pass 1 (V) + mask pass 2 (S)
            pb1 = stage_b1(prev) if prev is not None else None

            ox = wpool.tile([128, P], dt.float32, tag="ox")
            nc.vector.scalar_tensor_tensor(ox[:], bv[:], nhyc, t1t[:], A.mult, A.add)
            oy = wpool.tile([128, P], dt.float32, tag="oy")
            nc.vector.scalar_tensor_tensor(oy[:], bv[:], hxc, t2t[:], A.mult, A.add)

            dc = wpool.tile([128, P], dt.int16, tag="dc")
            sh = wpool.tile([128, P], dt.int32, tag="sh")
            for (ov, bc, b0T, resI) in ((ox, bxT, x0fT, dc), (oy, byT, y0fT, sh)):
                pw = wpool.tile([128, P], dt.float32, tag="pw")
                nc.vector.tensor_scalar(pw[:], ov[:], invdx[:], bc[:, it:it + 1],
                                        A.mult, A.add)
                ci = wpool.tile([128, P], dt.int32, tag="ci")
                nc.vector.tensor_copy(ci[:], pw[:])
                cf = wpool.tile([128, P], dt.float32, tag="cf")
                nc.vector.tensor_copy(cf[:], ci[:])
                ad = wpool.tile([128, P], dt.float32, tag="ad")
                nc.vector.tensor_tensor(ad[:], cf[:], pw[:], A.is_gt)
                nc.vector.scalar_tensor_tensor(cf[:], cf[:], b0T[:, it:it + 1],
                                               ad[:], A.subtract, A.subtract)
                nc.vector.tensor_copy(resI[:], cf[:])

            # window indices, all in fp32 (AP scalars must be fp32)
            gif = wpool.tile([128, NG], dt.float32, tag="gif")
            nc.vector.tensor_scalar(gif[:], iota7[:], G0fT[:, it:it + 1], None, A.add)
            x4f = wpool.tile([128, NG], dt.float32, tag="x4f")
            nc.vector.tensor_scalar(x4f[:], gif[:], 0.0625, None, A.mult)
            gci = wpool.tile([128, NG], dt.int32, tag="gci")
            nc.vector.tensor_copy(gci[:], x4f[:])
            gcf = wpool.tile([128, NG], dt.float32, tag="gcf")
            nc.vector.tensor_copy(gcf[:], gci[:])
            gad = wpool.tile([128, NG], dt.float32, tag="gad")
            nc.vector.tensor_tensor(gad[:], gcf[:], x4f[:], A.is_gt)
            nc.vector.tensor_tensor(x4f[:], gcf[:], gad[:], A.subtract)
            jf = wpool.tile([128, NG], dt.float32, tag="jf")
            nc.vector.scalar_tensor_tensor(jf[:], x4f[:], -16.0, gif[:], A.mult, A.add)
            j16 = wpool.tile([128, NG], dt.int16, tag="j16")
            nc.vector.tensor_copy(j16[:], jf[:])
            nc.vector.tensor_scalar(x4f[:], x4f[:], 86.0, q0mfT[:, it:it + 1],
                                    A.mult, A.add)
            e16 = wpool.tile([128, NG], dt.int16, tag="e16")
            nc.vector.tensor_copy(e16[:], x4f[:])

            g1 = gpool.tile([128, NG * 16 * 8], dt.float32, tag="g1")
            nc.gpsimd.ap_gather(g1[:], tab[:], e16[:], channels=128,
                                num_elems=2752, d=8, num_idxs=16 * NG)
            g2 = gpool.tile([128, NG * 16 * 8], dt.float32, tag="g2")
            nc.gpsimd.ap_gather(g2[:], mt2[:], j16[:], channels=128,
                                num_elems=16, d=8, num_idxs=16 * NG)

            nc.vector.tensor_tensor(g1[:], g1[:], g2[:], A.mult)

            # previous tile's select/extract/key (waits on the scalar mask pass)
            if pb1 is not None:
                stage_b2(pb1)

            pt = ppool.tile([128, NG * 8], dt.float32, tag="pt")
            mv4 = g1[:].rearrange("p (i k d) -> p k i d", i=NG, k=16, d=8)
            for k in range(16):
                nc.tensor.matmul(pt[:], st3[:, k, :], mv4[:, k, :, :],
                                 start=(k == 0), stop=(k == 15))
            prev = (it, pt, dc, sh)

        stage_b2(stage_b1(prev))

        # ---- epilogue: penalty from max-key ----
        eg = cpool.tile([128, NT * 4], dt.float32)
        ev = eg[:].rearrange("p (c t) -> p c t", c=4)
        d2m, es0, er, val = ev[:, 0, :], ev[:, 1, :], ev[:, 2, :], ev[:, 3, :]
        nc.vector.tensor_scalar(d2m, res[:], -1.0, 16.0, A.mult, A.add)
        nc.scalar.activation(es0, d2m, mybir.ActivationFunctionType.Sqrt)
        nc.vector.reciprocal(er, es0)
        nc.vector.tensor_tensor(er, d2m, er, A.mult)
        nc.vector.tensor_tensor(er, er, es0, A.add)
        nc.vector.tensor_scalar(er, er, 0.5, None, A.mult)
        nc.vector.tensor_tensor(er, er, invpenT[:], A.mult)
        nc.vector.tensor_scalar(er, er, -1.0, 1.0, A.mult, A.add)
        nc.vector.tensor_scalar(val, res[:], 0.0, None, A.is_gt)
        out_t = cpool.tile([128, NT], dt.float32)
        nc.vector.tensor_tensor(out_t[:], er, val, A.mult)
        nc.sync.dma_start(out_dram, out_t[:])

    nc.compile()
    return nc


def kernel(traj, veh_att, raster, mapixes, dx, _trace=False):
    _install_ntff_hook()
    from concourse.bass_utils import run_bass_kernel_spmd

    traj = np.ascontiguousarray(traj, np.float32)
    veh_att = np.ascontiguousarray(veh_att, np.float32)
    raster = np.ascontiguousarray(raster, np.float32)
    mapixes = np.ascontiguousarray(mapixes).astype(np.int64)
    dxf = np.float32(np.asarray(dx).reshape(-1)[0])

    # ---- host layout prep ----
    # 24-row packed fp32 words: wm24[m, q, x], rows [24q, 24q+24)
    bits = (raster >= 0.5).astype(np.int32)
    bits = np.concatenate([bits, np.zeros((N_MAPS, 24 * 88 - MAP_H, MAP_W), np.int32)],
                          axis=1)
    wts = (1 << np.arange(24)).astype(np.int64)
    wm24 = (bits.reshape(N_MAPS, 88, 24, MAP_W).astype(np.int64)
            * wts[None, None, :, None]).sum(axis=2).astype(np.float32)  # [4,88,2048]

    # table: partition p owns 4-col groups G === p%16 (mod 16);
    # entry e = (x4l-2)*QB + q -> d = [c(4) x (w[q], w[q+1])]
    # x = 64*x4l + 4*(p%16) + c;  x4l in [2, 29] -> NE = 28*86 = 2408 (pad 2752)
    tabs = []
    x4l = np.arange(2, 30)          # 28 values
    qq = np.arange(QB)
    cc = np.arange(4)
    ss = np.arange(2)
    for m in range(N_MAPS):
        t = np.zeros((128, 2752, 8), np.float32)
        for p in range(128):
            x = 64 * x4l[:, None, None, None] + 4 * (p % 16) + cc[None, None, :, None]
            q = qq[None, :, None, None] + ss[None, None, None, :]
            v = wm24[m, q, x]                        # [28, 86, 4, 2]
            t[p, :28 * QB] = v.reshape(28 * QB, 8)
        tabs.append(t.reshape(128, 2752 * 8))

    # one-hot (by group-partition) mask table
    mt2 = np.zeros((128, 16, 8), np.float32)
    mt2[np.arange(128), np.arange(128) % 16, :] = 1
    mt2 = mt2.reshape(128, 128)

    # 16 one-hot stationaries: stat[p, 128k + 16*(p//16)+k] = 1
    stat = np.zeros((128, 16, 128), np.float32)
    pp = np.arange(128)
    for k in range(16):
        stat[pp, k, 16 * (pp // 16) + k] = 1
    stat = stat.reshape(128, 16 * 128)

    uu2, vv2 = np.meshgrid(_UU10, _VV20, indexing="ij")
    uu_rep = np.broadcast_to(uu2.reshape(1, P), (128, P)).astype(np.float32).copy()
    vv_rep = np.broadcast_to(vv2.reshape(1, P), (128, P)).astype(np.float32).copy()
    uusq = (uu_rep * uu_rep).astype(np.float32)
    vvsq = (vv_rep * vv_rep).astype(np.float32)
    iota7 = np.broadcast_to(np.arange(NG, dtype=np.float32)[None, :], (128, NG)).copy()
    iota28 = np.broadcast_to(np.arange(NC, dtype=np.int16)[None, :], (128, NC)).copy()
    dxrep = np.full((128, 1), dxf, np.float32)

    # ---- shard agents by map, 2 cores per map ----
    core_agents = [[] for _ in range(N_CORES)]
    for m in range(N_MAPS):
        ags = np.where(mapixes == m)[0]
        half = (len(ags) + 1) // 2
        core_agents[2 * m] = list(ags[:half])
        core_agents[2 * m + 1] = list(ags[half:])

    n_states = [len(a) * T for a in core_agents]
    n_tiles = max(1, int(np.ceil(max(n_states) / 128)))
    S = n_tiles * 128

    traj_flat = traj.reshape(NA * T, 4)
    in_maps = []
    state_maps = []
    for c in range(N_CORES):
        ags = core_agents[c]
        tr = np.zeros((S, 4), np.float32)
        at = np.zeros((S, 2), np.float32)
        smap = np.full(S, -1, np.int64)
        if ags:
            idx = np.array([(a * T + t) for a in ags for t in range(T)])
            tr[:len(idx)] = traj_flat[idx]
            at[:len(idx)] = veh_att[np.repeat(ags, T)]
            smap[:len(idx)] = idx
        pad = smap < 0
        tr[pad] = np.array([100.0, 100.0, 1.0, 0.0], np.float32)
        at[pad] = np.array([4.0, 2.0], np.float32)
        trt = tr.reshape(n_tiles, 128, 4).transpose(1, 0, 2)
        att2 = at.reshape(n_tiles, 128, 2).transpose(1, 0, 2)
        in_maps.append({
            "tab": tabs[c // 2], "mt2": mt2, "stat": stat,
            "uu": uu_rep, "vv": vv_rep, "uusq": uusq, "vvsq": vvsq,
            "iota7": iota7, "iota28": iota28, "dxrep": dxrep,
            "cxs": np.ascontiguousarray(trt[:, :, 0]),
            "cys": np.ascontiguousarray(trt[:, :, 1]),
            "hxs": np.ascontiguousarray(trt[:, :, 2]),
            "hys": np.ascontiguousarray(trt[:, :, 3]),
            "Ls": np.ascontiguousarray(att2[:, :, 0]),
            "Ws": np.ascontiguousarray(att2[:, :, 1]),
        })
        state_maps.append(smap)

    if n_tiles not in _PROGRAM_CACHE:
        _PROGRAM_CACHE[n_tiles] = _build_program(n_tiles)
    nc = _PROGRAM_CACHE[n_tiles]

    try:
        res = run_bass_kernel_spmd(nc, in_maps, list(range(N_CORES)), trace=_trace)
    except Exception:
        if not _trace:
            raise
        res = run_bass_kernel_spmd(nc, in_maps, list(range(N_CORES)), trace=False)
    kernel.last_results = res

    out = np.zeros(NA * T, np.float32)
    for c in range(N_CORES):
        o = res.results[c]["outsh"].T.reshape(-1)
        valid = state_maps[c] >= 0
        out[state_maps[c][valid]] = o[valid]
    return out
